# revision 2
# baseline (speedup 1.0000x reference)
"""Binarized linear block (y = relu(batchnorm(x @ sign(W).T))) on 8 TRN2 NeuronCores.

Strategy:
  - Data-parallel shard of the batch dim (16384 -> 2048 rows/core).
  - Weights binarized + transposed + tiled on host, replicated to all cores
    as fp16 (+-1 is exact in fp16).
  - x is cast to fp16 on host and pre-transposed so the contraction dim (IN)
    lies on SBUF partitions; the matmul computes y^T tiles [o x b] so the
    BN batch statistics are per-partition free-dim reductions.
  - The BN batch MEAN is computed exactly on the host (colsum(x16) @ sign(W)
    is cheap) and shipped as an input; only the per-channel second moment
    needs the cross-core exchange.
  - Sync-BN second moments go through tiny per-group AllGathers over DRAM
    bounce buffers, software-pipelined against the matmul stream.  All
    collective-adjacent work (bounce DMAs, gathered-stat reduction, scale/
    shift algebra) runs on the otherwise-idle GpSimd queue so it can never
    head-of-line block the PSUM-draining ScalarE/VectorE queues; the few ops
    that must run on ScalarE/VectorE/PE (sqrt, reciprocal, PSUM reads, PE
    transposes) are order-pinned behind much-later stream ops so their
    collective-dependent waits are satisfied before the engines reach them.
  - Startup: the first weight tile and first x chunk are DMA'd in k-slices
    so the first matmul can start as soon as ~0.75 MB has landed.
  - Output is written as y^T tiles [m, 128, b_loc]; host transposes back.
"""

import numpy as np

_BN_EPS = 1e-5

_CACHE = {}


def _env(name):
    import os

    return bool(os.environ.get(name))


def _group_sizes(mt):
    # Pipelined sync-BN in small uniform groups. Each group's collective is
    # triggered right after its matmuls; its finish phase is emitted three
    # collectives later, so the collective has several group-times of matmul
    # compute to land before any engine reaches the finish instructions.
    if mt <= 2:
        return [mt]
    if mt <= 4:
        return [mt - 2, 1, 1]
    rest = mt - 2
    return [2] * (rest // 2) + ([1] if rest % 2 else []) + [1, 1]


def _build(n_cores, b_loc, in_dim, out_dim, b_total):
    import concourse.bass as bass  # noqa: F401
    import concourse.mybir as mybir
    import concourse.tile as tile
    from concourse import bacc

    f16 = mybir.dt.bfloat16 if _env("KBN_BF16") else mybir.dt.float16
    f32 = mybir.dt.float32
    AF = mybir.ActivationFunctionType
    ALU = mybir.AluOpType

    KT = in_dim // 128   # k tiles (contraction)
    MT = out_dim // 128  # output-channel tiles
    CH = min(512, b_loc)  # moving-operand chunk
    NCH = b_loc // CH    # batch chunks
    groups = _group_sizes(MT)

    nc = bacc.Bacc(
        "TRN2",
        target_bir_lowering=False,
        debug=False,
        enable_asserts=False,
        num_devices=n_cores,
    )

    # xt[p, n, k, b] so each batch-chunk DMA is contiguous per partition
    xt = nc.dram_tensor("xt", [128, NCH, KT, CH], f16, kind="ExternalInput")
    wt = nc.dram_tensor("wt", [MT, 128, KT, 128], f16, kind="ExternalInput")
    gmt = nc.dram_tensor("gmt", [128, MT], f32, kind="ExternalInput")
    bta = nc.dram_tensor("bta", [128, MT], f32, kind="ExternalInput")
    # exact batch mean (host-computed) and its square, in [128, MT] layout
    mut = nc.dram_tensor("mut", [128, MT], f32, kind="ExternalInput")
    mu2 = nc.dram_tensor("mu2", [128, MT], f32, kind="ExternalInput")
    out = nc.dram_tensor("out", [MT, 128, b_loc], f32, kind="ExternalOutput")

    with tile.TileContext(nc) as tc:
        with (
            tc.tile_pool(name="xpool", bufs=1) as xpool,
            tc.tile_pool(name="wpool", bufs=4) as wpool,
            tc.tile_pool(name="ypool", bufs=MT) as ypool,
            tc.tile_pool(name="opool", bufs=4) as opool,
            tc.tile_pool(name="stat", bufs=1) as stat,
            tc.tile_pool(name="gstat", bufs=4) as gstat,
            tc.tile_pool(name="psum", bufs=4, space="PSUM") as psum,
            tc.tile_pool(name="psum2", bufs=2, space="PSUM") as psum2,
            tc.tile_pool(name="dram", bufs=4, space="DRAM") as dram,
        ):
            xt_sb = xpool.tile([128, NCH, KT, CH], f16)
            # Startup: k-sliced loads so the first matmuls are gated on
            # ~0.75MB instead of ~2MB.  First weight tile in k-halves, first
            # x chunk in k-quarters, interleaved by need-order.
            kq = max(KT // 4, 1)
            kh = max(KT // 2, 1)
            wts = []
            wt_0 = wpool.tile([128, KT, 128], f16, tag="wt")
            nc.sync.dma_start(wt_0[:, :kh], wt.ap()[0, :, :kh])
            nc.sync.dma_start(xt_sb[:, 0, :kq], xt.ap()[:, 0, :kq])
            nc.sync.dma_start(xt_sb[:, 0, kq : 2 * kq], xt.ap()[:, 0, kq : 2 * kq])
            nc.sync.dma_start(wt_0[:, kh:], wt.ap()[0, :, kh:])
            nc.sync.dma_start(xt_sb[:, 0, 2 * kq :], xt.ap()[:, 0, 2 * kq :])
            wts.append(wt_0)
            if MT > 1:
                wt_1 = wpool.tile([128, KT, 128], f16, tag="wt")
                nc.sync.dma_start(wt_1[:], wt.ap()[1])
                wts.append(wt_1)
            for n in range(1, NCH):
                nc.sync.dma_start(xt_sb[:, n], xt.ap()[:, n])

            gamma_sb = stat.tile([128, MT], f32)
            beta_sb = stat.tile([128, MT], f32)
            mu_sb = stat.tile([128, MT], f32)
            mu2_sb = stat.tile([128, MT], f32)
            nc.gpsimd.dma_start(gamma_sb[:], gmt.ap())
            nc.gpsimd.dma_start(beta_sb[:], bta.ap())
            nc.gpsimd.dma_start(mu_sb[:], mut.ap())
            nc.gpsimd.dma_start(mu2_sb[:], mu2.ap())

            eps_t = stat.tile([128, 1], f32)
            nc.vector.memset(eps_t[:], _BN_EPS)

            # identity for PE-based transposes of the tiny stats tensors:
            # a [128, c] SBUF->DRAM DMA is 128 c*4B descriptors (~20us even
            # on HWDGE), while the [c, 128] transposed layout is c 512B
            # descriptors. The two transposes cost ~600ns of PE each.
            from concourse.masks import make_identity

            ident = stat.tile([128, 128], f32)
            make_identity(nc, ident[:])

            yts = [None] * MT
            last_mm = [None]    # most recent matmul instruction
            last_act = [None]   # most recent PSUM-drain ACTIVATE (ScalarE)
            last_stat = [None]  # most recent bn_stats (VectorE)

            def pin(inst, anchor):
                # order-only (no semaphore) same-engine pin: keeps
                # collective-dependent ops from being scheduled ahead of
                # stream work on the strict-FIFO engine queues
                if anchor[0] is not None:
                    tile.add_dep_helper(
                        inst.ins,
                        anchor[0].ins,
                        sync=False,
                        reason="pin collective-dependent op behind stream",
                    )

            def emit_chunk(m, wt_m, bns, j, n):
                """One (channel-tile, batch-chunk): 16 matmuls + epilogues."""
                ns = slice(n * CH, (n + 1) * CH)
                ps = psum.tile([128, CH], f32)
                for k in range(KT):
                    last_mm[0] = nc.tensor.matmul(
                        ps[:],
                        wt_m[:, k, :],
                        xt_sb[:, n, k, :],
                        start=(k == 0),
                        stop=(k == KT - 1),
                    )
                # ScalarE: fp16 copy of y^T; VectorE: batch stats
                last_act[0] = nc.scalar.activation(yts[m][:, ns], ps[:], AF.Identity)
                last_stat[0] = nc.vector.bn_stats(out=bns[:, j, n, :], in_=ps[:])

            def emit_collective(m0, gm, bns):
                """Pack the group's second moments and launch its AllGather."""
                # local (mean, var) per channel tile in the group
                mv = gstat.tile([128, gm, 2], f32, tag="mv")
                for j in range(gm):
                    nc.vector.bn_aggr(out=mv[:, j, :], in_=bns[:, j])

                # per-core second moment m2 = var + mean^2, PE-transposed to
                # [gm, 128] so the bounce DMA is gm big descriptors
                st = gstat.tile([128, gm], f32, tag="st")
                nc.vector.tensor_mul(st[:], mv[:, :, 0], mv[:, :, 0])
                nc.vector.tensor_add(st[:], mv[:, :, 1], st[:])

                psT = psum2.tile([gm, 128], f32, tag="psT")
                nc.tensor.transpose(psT[:], st[:], ident[:])
                stT = gstat.tile([gm, 128], f32, tag="stT")
                nc.vector.tensor_copy(stT[:], psT[:])

                bounce_out = None
                if n_cores > 1:
                    bounce_in = dram.tile([gm, 128], f32, tag="bin")
                    nc.gpsimd.dma_start(bounce_in[:], stT[:])
                    # AllGather + local reduce: lower latency than an
                    # AllReduce for latency-dominated tiny messages
                    bounce_out = dram.tile([n_cores, gm, 128], f32, tag="bout")
                    nc.gpsimd.collective_compute(
                        "AllGather",
                        ALU.bypass,
                        replica_groups=[list(range(n_cores))],
                        ins=[bounce_in.opt()],
                        outs=[bounce_out.opt()],
                    )
                return (m0, gm, bounce_out, stT)

            def emit_finish(state, split=1):
                m0, gm, bounce_out, stT = state
                if n_cores > 1:
                    allT = gstat.tile([gm, n_cores, 128], f32, tag="allT")
                    nc.gpsimd.dma_start(
                        allT[:], bounce_out[:].rearrange("r c f -> c r f")
                    )
                    w = n_cores
                    while w > 1:
                        w //= 2
                        nc.gpsimd.tensor_add(
                            allT[:, :w], allT[:, :w], allT[:, w : 2 * w]
                        )
                    redT = allT[:, 0, :]
                else:
                    redT = stT[:]
                psB = psum2.tile([128, gm], f32, tag="psB")
                back_t = nc.tensor.transpose(psB[:], redT, ident[:gm, :gm])
                # pin behind the newest matmul so this collective-dependent
                # PE op can't head-of-line block the matmul stream
                pin(back_t, last_mm)
                m2g = gstat.tile([128, gm], f32, tag="m2g")
                cpy = nc.vector.tensor_copy(m2g[:], psB[:])
                pin(cpy, last_stat)

                ms = slice(m0, m0 + gm)
                var_t = gstat.tile([128, gm], f32, tag="var")
                sd_t = gstat.tile([128, gm], f32, tag="sd")
                inv_t = gstat.tile([128, gm], f32, tag="inv")
                scale_t = gstat.tile([128, gm], f32, tag="scale")
                tmp_t = gstat.tile([128, gm], f32, tag="tmp")
                shift_t = gstat.tile([128, gm], f32, tag="shift")
                # var = m2/n_cores - mu^2   (exact mean from host)
                nc.gpsimd.tensor_scalar_mul(var_t[:], m2g[:], 1.0 / n_cores)
                nc.gpsimd.tensor_sub(var_t[:], var_t[:], mu2_sb[:, ms])
                sq = nc.scalar.activation(sd_t[:], var_t[:], AF.Sqrt, bias=eps_t[:])
                pin(sq, last_act)
                rc = nc.vector.reciprocal(inv_t[:], sd_t[:])
                pin(rc, last_stat)
                nc.gpsimd.tensor_mul(scale_t[:], gamma_sb[:, ms], inv_t[:])
                nc.gpsimd.tensor_mul(tmp_t[:], mu_sb[:, ms], scale_t[:])
                nc.gpsimd.tensor_sub(shift_t[:], beta_sb[:, ms], tmp_t[:])

                hb = b_loc // split
                for j, m in enumerate(range(m0, m0 + gm)):
                    out_m = opool.tile([128, b_loc], f32)
                    for h in range(split):
                        hs = slice(h * hb, (h + 1) * hb)
                        act = nc.scalar.activation(
                            out_m[:, hs],
                            yts[m][:, hs],
                            AF.Relu,
                            bias=shift_t[:, j : j + 1],
                            scale=scale_t[:, j : j + 1],
                        )
                        pin(act, last_act)
                        nc.sync.dma_start(out.ap()[m, :, hs], out_m[:, hs])

            # Pipeline: batch-chunk-outer within each group (halves the
            # cold-start HBM demand so the PE doesn't outrun the x DMAs).
            # Group g's collective block is emitted after group g+1's first
            # chunk pass; its finish is emitted three collectives later. The
            # last two groups are single-tile with immediately-emitted
            # collectives, spaced a whole m-tile apart so they don't queue on
            # the serial CC stream: the final exposed AllGather is the 1-tile
            # one, and its finish critical path is kept lean.
            states = []
            pend_coll = None
            m0 = 0
            G = len(groups)
            for g, gm in enumerate(groups):
                ms = list(range(m0, m0 + gm))
                bns = gstat.tile([128, gm, NCH, 6], f32, tag="bns")
                wtiles = []
                for m in ms:
                    if m < len(wts):
                        wtiles.append(wts[m])
                    else:
                        wt_m = wpool.tile([128, KT, 128], f16, tag="wt")
                        nc.sync.dma_start(wt_m[:], wt.ap()[m])
                        wtiles.append(wt_m)
                    yt_m = ypool.tile([128, b_loc], f16, tag="yt")
                    yts[m] = yt_m
                for n in range(NCH):
                    for j, m in enumerate(ms):
                        emit_chunk(m, wtiles[j], bns, j, n)
                    if n == 0 and pend_coll is not None:
                        states.append(emit_collective(*pend_coll))
                        pend_coll = None
                        if len(states) > 3:
                            emit_finish(states.pop(0))
                if g >= G - 2:
                    states.append(emit_collective(m0, gm, bns))
                    if len(states) > 3:
                        emit_finish(states.pop(0))
                else:
                    pend_coll = (m0, gm, bns)
                m0 += gm
            if pend_coll is not None:
                states.append(emit_collective(*pend_coll))
            for state in states:
                emit_finish(state, split=2)

    nc.compile()
    return nc


def _get_nc(key):
    if key not in _CACHE:
        _CACHE[key] = _build(*key)
    return _CACHE[key]


def _prepare_in_maps(x, weight, gamma, beta, n_cores):
    b_total, in_dim = x.shape
    out_dim = weight.shape[0]
    b_loc = b_total // n_cores
    KT = in_dim // 128
    MT = out_dim // 128
    CH = min(512, b_loc)
    NCH = b_loc // CH

    # host-side marshalling (binarize / transpose / cast / tile)
    wb = np.where(weight >= 0, np.float32(1.0), np.float32(-1.0))
    # wt[m, p, k, o] = sign(W)[m*128+o, k*128+p]
    import ml_dtypes as _mld

    _hdt = _mld.bfloat16 if _env("KBN_BF16") else np.float16
    wt = np.ascontiguousarray(
        wb.reshape(MT, 128, KT, 128).transpose(0, 3, 2, 1).astype(_hdt)
    )
    gmt = np.ascontiguousarray(gamma.reshape(MT, 128).T.astype(np.float32))
    bta = np.ascontiguousarray(beta.reshape(MT, 128).T.astype(np.float32))

    x16 = x.astype(_hdt)

    # exact batch mean of the device's y (built from the quantized x the
    # kernel actually uses): mean = colsum(x16) @ sign(W)^T / B
    colsum = x16.astype(np.float64).sum(axis=0)
    mu = (wb.astype(np.float64) @ colsum) / b_total  # [out]
    mut = np.ascontiguousarray(
        mu.reshape(MT, 128).T.astype(np.float32)
    )
    mu2 = np.ascontiguousarray(
        (mu * mu).reshape(MT, 128).T.astype(np.float32)
    )

    in_maps = []
    for c in range(n_cores):
        xc = x16[c * b_loc : (c + 1) * b_loc]  # [b, in]
        # xt[p, n, k, b] = x[b0 + n*CH + b, k*128+p]
        xt = np.ascontiguousarray(
            xc.reshape(NCH, CH, KT, 128).transpose(3, 0, 2, 1)
        )
        in_maps.append(
            {"xt": xt, "wt": wt, "gmt": gmt, "bta": bta, "mut": mut, "mu2": mu2}
        )
    return in_maps


def _gather_out(results, b_total, out_dim, n_cores):
    b_loc = b_total // n_cores
    out = np.empty((b_total, out_dim), dtype=np.float32)
    for c in range(n_cores):
        oc = np.asarray(results[c]["out"]).reshape(out_dim // 128, 128, b_loc)
        out[c * b_loc : (c + 1) * b_loc] = oc.transpose(2, 0, 1).reshape(
            b_loc, out_dim
        )
    return out


def kernel(x, weight, gamma, beta):
    from concourse.bass_utils import run_bass_kernel_spmd

    n_cores = 8
    b_total, in_dim = x.shape
    out_dim = weight.shape[0]

    nc = _get_nc((n_cores, b_total // n_cores, in_dim, out_dim, b_total))
    in_maps = _prepare_in_maps(x, weight, gamma, beta, n_cores)
    res = run_bass_kernel_spmd(nc, in_maps, list(range(n_cores)))
    return _gather_out(res.results, b_total, out_dim, n_cores)


# revision 6
# speedup vs baseline: 1.0254x; 1.0254x over previous
"""Binarized linear block (y = relu(batchnorm(x @ sign(W).T))) on 8 TRN2 NeuronCores.

Strategy:
  - Data-parallel shard of the batch dim (16384 -> 2048 rows/core).
  - Weights binarized + transposed + tiled on host, replicated to all cores
    as fp16 (+-1 is exact in fp16).
  - x is cast to fp16 on host and pre-transposed so the contraction dim (IN)
    lies on SBUF partitions; the matmul computes y^T tiles [o x b] so the
    BN batch statistics are per-partition free-dim reductions.
  - The BN batch MEAN is computed exactly on the host (colsum(x16) @ sign(W)
    is cheap) and shipped as an input; only the per-channel second moment
    needs the cross-core exchange.
  - Sync-BN second moments go through tiny per-group AllGathers over DRAM
    bounce buffers, software-pipelined against the matmul stream.  All
    collective-adjacent work (bounce DMAs, gathered-stat reduction, scale/
    shift algebra) runs on the otherwise-idle GpSimd queue so it can never
    head-of-line block the PSUM-draining ScalarE/VectorE queues; the few ops
    that must run on ScalarE/VectorE/PE (sqrt, reciprocal, PSUM reads, PE
    transposes) are order-pinned behind much-later stream ops so their
    collective-dependent waits are satisfied before the engines reach them.
  - Startup: the first weight tile and first x chunk are DMA'd in k-slices
    so the first matmul can start as soon as ~0.75 MB has landed.
  - Output is written as y^T tiles [m, 128, b_loc]; host transposes back.
"""

import numpy as np

_BN_EPS = 1e-5

_CACHE = {}


def _env(name):
    import os

    return bool(os.environ.get(name))


def _group_sizes(mt):
    # Pipelined sync-BN in small uniform groups. Each group's collective is
    # triggered right after its matmuls; its finish phase is emitted three
    # collectives later, so the collective has several group-times of matmul
    # compute to land before any engine reaches the finish instructions.
    if mt <= 2:
        return [mt]
    if mt <= 4:
        return [mt - 2, 1, 1]
    rest = mt - 2
    return [2] * (rest // 2) + ([1] if rest % 2 else []) + [1, 1]


def _build(n_cores, b_loc, in_dim, out_dim, b_total):
    import concourse.bass as bass  # noqa: F401
    import concourse.mybir as mybir
    import concourse.tile as tile
    from concourse import bacc

    f16 = mybir.dt.bfloat16 if _env("KBN_BF16") else mybir.dt.float16
    f32 = mybir.dt.float32
    AF = mybir.ActivationFunctionType
    ALU = mybir.AluOpType

    KT = in_dim // 128   # k tiles (contraction)
    MT = out_dim // 128  # output-channel tiles
    CH = min(512, b_loc)  # moving-operand chunk
    NCH = b_loc // CH    # batch chunks
    groups = _group_sizes(MT)

    nc = bacc.Bacc(
        "TRN2",
        target_bir_lowering=False,
        debug=False,
        enable_asserts=False,
        num_devices=n_cores,
    )

    # xt[p, n, k, b] so each batch-chunk DMA is contiguous per partition
    xt = nc.dram_tensor("xt", [128, NCH, KT, CH], f16, kind="ExternalInput")
    wt = nc.dram_tensor("wt", [MT, 128, KT, 128], f16, kind="ExternalInput")
    gmt = nc.dram_tensor("gmt", [128, MT], f32, kind="ExternalInput")
    bta = nc.dram_tensor("bta", [128, MT], f32, kind="ExternalInput")
    # exact batch mean (host-computed) and its square, in [128, MT] layout
    mut = nc.dram_tensor("mut", [128, MT], f32, kind="ExternalInput")
    mu2 = nc.dram_tensor("mu2", [128, MT], f32, kind="ExternalInput")
    out = nc.dram_tensor("out", [MT, 128, b_loc], f32, kind="ExternalOutput")

    with tile.TileContext(nc) as tc:
        with (
            tc.tile_pool(name="xpool", bufs=1) as xpool,
            tc.tile_pool(name="wpool", bufs=4) as wpool,
            tc.tile_pool(name="ypool", bufs=MT) as ypool,
            tc.tile_pool(name="opool", bufs=4) as opool,
            tc.tile_pool(name="stat", bufs=1) as stat,
            tc.tile_pool(name="gstat", bufs=4) as gstat,
            tc.tile_pool(name="psum", bufs=4, space="PSUM") as psum,
            tc.tile_pool(name="psum2", bufs=2, space="PSUM") as psum2,
            tc.tile_pool(name="dram", bufs=4, space="DRAM") as dram,
        ):
            xt_sb = xpool.tile([128, NCH, KT, CH], f16)
            # Startup: k-sliced loads so the first matmuls are gated on
            # ~0.75MB instead of ~2MB.  First weight tile in k-halves, first
            # x chunk in k-quarters, interleaved by need-order; chunk 1
            # before the second weight tile (the m0/m1 chunk pass consumes
            # chunks faster than weight tiles).
            kq = max(KT // 4, 1)
            kh = max(KT // 2, 1)
            wts = []
            wt_0 = wpool.tile([128, KT, 128], f16, tag="wt")
            nc.sync.dma_start(wt_0[:, :kh], wt.ap()[0, :, :kh])
            nc.sync.dma_start(xt_sb[:, 0, :kq], xt.ap()[:, 0, :kq])
            nc.sync.dma_start(xt_sb[:, 0, kq : 2 * kq], xt.ap()[:, 0, kq : 2 * kq])
            nc.sync.dma_start(xt_sb[:, 0, 2 * kq : 3 * kq], xt.ap()[:, 0, 2 * kq : 3 * kq])
            nc.sync.dma_start(wt_0[:, kh:], wt.ap()[0, :, kh:])
            nc.sync.dma_start(xt_sb[:, 0, 3 * kq :], xt.ap()[:, 0, 3 * kq :])
            if NCH > 1:
                nc.sync.dma_start(xt_sb[:, 1], xt.ap()[:, 1])
            wts.append(wt_0)
            if MT > 1:
                wt_1 = wpool.tile([128, KT, 128], f16, tag="wt")
                nc.sync.dma_start(wt_1[:], wt.ap()[1])
                wts.append(wt_1)
            for n in range(2, NCH):
                nc.sync.dma_start(xt_sb[:, n], xt.ap()[:, n])

            gamma_sb = stat.tile([128, MT], f32)
            beta_sb = stat.tile([128, MT], f32)
            mu_sb = stat.tile([128, MT], f32)
            mu2_sb = stat.tile([128, MT], f32)
            nc.gpsimd.dma_start(gamma_sb[:], gmt.ap())
            nc.gpsimd.dma_start(beta_sb[:], bta.ap())
            nc.gpsimd.dma_start(mu_sb[:], mut.ap())
            nc.gpsimd.dma_start(mu2_sb[:], mu2.ap())

            eps_t = stat.tile([128, 1], f32)
            nc.vector.memset(eps_t[:], _BN_EPS)
            # nb = eps - mu^2: lets single-tile groups fold the whole
            # variance computation into the Sqrt activation's bias
            nb_t = stat.tile([128, MT], f32)
            nc.gpsimd.memset(nb_t[:], _BN_EPS)
            nc.gpsimd.tensor_sub(nb_t[:], nb_t[:], mu2_sb[:])

            # identity for PE-based transposes of the tiny stats tensors:
            # a [128, c] SBUF->DRAM DMA is 128 c*4B descriptors (~20us even
            # on HWDGE), while the [c, 128] transposed layout is c 512B
            # descriptors. The two transposes cost ~600ns of PE each.
            from concourse.masks import make_identity

            ident = stat.tile([128, 128], f32)
            make_identity(nc, ident[:])

            yts = [None] * MT
            last_mm = [None]    # most recent matmul instruction
            last_act = [None]   # most recent PSUM-drain ACTIVATE (ScalarE)
            last_stat = [None]  # most recent bn_stats (VectorE)

            def pin(inst, anchor):
                # order-only (no semaphore) same-engine pin: keeps
                # collective-dependent ops from being scheduled ahead of
                # stream work on the strict-FIFO engine queues
                if anchor[0] is not None:
                    tile.add_dep_helper(
                        inst.ins,
                        anchor[0].ins,
                        sync=False,
                        reason="pin collective-dependent op behind stream",
                    )

            def emit_chunk(m, wt_m, bns, j, n):
                """One (channel-tile, batch-chunk): 16 matmuls + epilogues."""
                ns = slice(n * CH, (n + 1) * CH)
                ps = psum.tile([128, CH], f32)
                for k in range(KT):
                    last_mm[0] = nc.tensor.matmul(
                        ps[:],
                        wt_m[:, k, :],
                        xt_sb[:, n, k, :],
                        start=(k == 0),
                        stop=(k == KT - 1),
                    )
                # ScalarE: fp16 copy of y^T; VectorE: batch stats
                last_act[0] = nc.scalar.activation(yts[m][:, ns], ps[:], AF.Identity)
                last_stat[0] = nc.vector.bn_stats(out=bns[:, j, n, :], in_=ps[:])

            def emit_collective(m0, gm, bns):
                """Pack the group's second moments and launch its AllGather."""
                # local (mean, var) per channel tile in the group
                mv = gstat.tile([128, gm, 2], f32, tag="mv")
                for j in range(gm):
                    nc.vector.bn_aggr(out=mv[:, j, :], in_=bns[:, j])

                # per-core second moment m2 = var + mean^2, PE-transposed to
                # [gm, 128] so the bounce DMA is gm big descriptors
                st = gstat.tile([128, gm], f32, tag="st")
                nc.vector.tensor_mul(st[:], mv[:, :, 0], mv[:, :, 0])
                nc.vector.tensor_add(st[:], mv[:, :, 1], st[:])

                psT = psum2.tile([gm, 128], f32, tag="psT")
                nc.tensor.transpose(psT[:], st[:], ident[:])
                stT = gstat.tile([gm, 128], f32, tag="stT")
                nc.vector.tensor_copy(stT[:], psT[:])

                bounce_out = None
                if n_cores > 1:
                    bounce_in = dram.tile([gm, 128], f32, tag="bin")
                    nc.gpsimd.dma_start(bounce_in[:], stT[:])
                    # AllGather + local reduce: lower latency than an
                    # AllReduce for latency-dominated tiny messages
                    bounce_out = dram.tile([n_cores, gm, 128], f32, tag="bout")
                    nc.gpsimd.collective_compute(
                        "AllGather",
                        ALU.bypass,
                        replica_groups=[list(range(n_cores))],
                        ins=[bounce_in.opt()],
                        outs=[bounce_out.opt()],
                    )
                return (m0, gm, bounce_out, stT)

            def emit_finish(state, split=1):
                m0, gm, bounce_out, stT = state
                if n_cores > 1:
                    allT = gstat.tile([gm, n_cores, 128], f32, tag="allT")
                    nc.gpsimd.dma_start(
                        allT[:], bounce_out[:].rearrange("r c f -> c r f")
                    )
                    w = n_cores
                    while w > 1:
                        w //= 2
                        ad = nc.vector.tensor_add(
                            allT[:, :w], allT[:, :w], allT[:, w : 2 * w]
                        )
                        pin(ad, last_stat)
                    redT = allT[:, 0, :]
                else:
                    redT = stT[:]
                psB = psum2.tile([128, gm], f32, tag="psB")
                back_t = nc.tensor.transpose(psB[:], redT, ident[:gm, :gm])
                # pin behind the newest matmul so this collective-dependent
                # PE op can't head-of-line block the matmul stream
                pin(back_t, last_mm)

                ms = slice(m0, m0 + gm)
                sd_t = gstat.tile([128, gm], f32, tag="sd")
                inv_t = gstat.tile([128, gm], f32, tag="inv")
                scale_t = gstat.tile([128, gm], f32, tag="scale")
                tmp_t = gstat.tile([128, gm], f32, tag="tmp")
                shift_t = gstat.tile([128, gm], f32, tag="shift")
                if gm == 1:
                    # lean path: sd = sqrt(m2/W + (eps - mu^2)) straight from
                    # PSUM, variance folded into the activation bias
                    sq = nc.scalar.activation(
                        sd_t[:],
                        psB[:],
                        AF.Sqrt,
                        bias=nb_t[:, m0 : m0 + 1],
                        scale=1.0 / n_cores,
                    )
                    pin(sq, last_act)
                else:
                    m2g = gstat.tile([128, gm], f32, tag="m2g")
                    cpy = nc.vector.tensor_copy(m2g[:], psB[:])
                    pin(cpy, last_stat)
                    var_t = gstat.tile([128, gm], f32, tag="var")
                    ts = nc.vector.tensor_scalar_mul(var_t[:], m2g[:], 1.0 / n_cores)
                    pin(ts, last_stat)
                    sb = nc.vector.tensor_sub(var_t[:], var_t[:], mu2_sb[:, ms])
                    pin(sb, last_stat)
                    sq = nc.scalar.activation(
                        sd_t[:], var_t[:], AF.Sqrt, bias=eps_t[:]
                    )
                    pin(sq, last_act)
                rc = nc.vector.reciprocal(inv_t[:], sd_t[:])
                pin(rc, last_stat)
                sc = nc.vector.tensor_mul(scale_t[:], gamma_sb[:, ms], inv_t[:])
                pin(sc, last_stat)
                tm = nc.vector.tensor_mul(tmp_t[:], mu_sb[:, ms], scale_t[:])
                pin(tm, last_stat)
                sh = nc.vector.tensor_sub(shift_t[:], beta_sb[:, ms], tmp_t[:])
                pin(sh, last_stat)

                hb = b_loc // split
                for j, m in enumerate(range(m0, m0 + gm)):
                    out_m = opool.tile([128, b_loc], f32)
                    for h in range(split):
                        hs = slice(h * hb, (h + 1) * hb)
                        act = nc.scalar.activation(
                            out_m[:, hs],
                            yts[m][:, hs],
                            AF.Relu,
                            bias=shift_t[:, j : j + 1],
                            scale=scale_t[:, j : j + 1],
                        )
                        pin(act, last_act)
                        nc.sync.dma_start(out.ap()[m, :, hs], out_m[:, hs])

            # Pipeline: batch-chunk-outer within each group (halves the
            # cold-start HBM demand so the PE doesn't outrun the x DMAs).
            # Group g's collective block is emitted after group g+1's first
            # chunk pass; its finish is emitted three collectives later. The
            # last two groups are single-tile with immediately-emitted
            # collectives, spaced a whole m-tile apart so they don't queue on
            # the serial CC stream: the final exposed AllGather is the 1-tile
            # one, and its finish critical path is kept lean.
            states = []
            pend_coll = None
            m0 = 0
            G = len(groups)
            for g, gm in enumerate(groups):
                ms = list(range(m0, m0 + gm))
                bns = gstat.tile([128, gm, NCH, 6], f32, tag="bns")
                wtiles = []
                for m in ms:
                    if m < len(wts):
                        wtiles.append(wts[m])
                    else:
                        wt_m = wpool.tile([128, KT, 128], f16, tag="wt")
                        nc.sync.dma_start(wt_m[:], wt.ap()[m])
                        wtiles.append(wt_m)
                    yt_m = ypool.tile([128, b_loc], f16, tag="yt")
                    yts[m] = yt_m
                for n in range(NCH):
                    for j, m in enumerate(ms):
                        emit_chunk(m, wtiles[j], bns, j, n)
                    if n == 0 and pend_coll is not None:
                        states.append(emit_collective(*pend_coll))
                        pend_coll = None
                        if len(states) > 2:
                            emit_finish(states.pop(0))
                if g >= G - 2:
                    states.append(emit_collective(m0, gm, bns))
                    if len(states) > 2:
                        emit_finish(states.pop(0), split=2)
                else:
                    pend_coll = (m0, gm, bns)
                m0 += gm
            if pend_coll is not None:
                states.append(emit_collective(*pend_coll))
            for state in states:
                emit_finish(state, split=2)

    nc.compile()
    return nc


def _get_nc(key):
    if key not in _CACHE:
        _CACHE[key] = _build(*key)
    return _CACHE[key]


def _prepare_in_maps(x, weight, gamma, beta, n_cores):
    b_total, in_dim = x.shape
    out_dim = weight.shape[0]
    b_loc = b_total // n_cores
    KT = in_dim // 128
    MT = out_dim // 128
    CH = min(512, b_loc)
    NCH = b_loc // CH

    # host-side marshalling (binarize / transpose / cast / tile)
    wb = np.where(weight >= 0, np.float32(1.0), np.float32(-1.0))
    # wt[m, p, k, o] = sign(W)[m*128+o, k*128+p]
    import ml_dtypes as _mld

    _hdt = _mld.bfloat16 if _env("KBN_BF16") else np.float16
    wt = np.ascontiguousarray(
        wb.reshape(MT, 128, KT, 128).transpose(0, 3, 2, 1).astype(_hdt)
    )
    gmt = np.ascontiguousarray(gamma.reshape(MT, 128).T.astype(np.float32))
    bta = np.ascontiguousarray(beta.reshape(MT, 128).T.astype(np.float32))

    x16 = x.astype(_hdt)

    # exact batch mean of the device's y (built from the quantized x the
    # kernel actually uses): mean = colsum(x16) @ sign(W)^T / B
    colsum = x16.astype(np.float64).sum(axis=0)
    mu = (wb.astype(np.float64) @ colsum) / b_total  # [out]
    mut = np.ascontiguousarray(
        mu.reshape(MT, 128).T.astype(np.float32)
    )
    mu2 = np.ascontiguousarray(
        (mu * mu).reshape(MT, 128).T.astype(np.float32)
    )

    in_maps = []
    for c in range(n_cores):
        xc = x16[c * b_loc : (c + 1) * b_loc]  # [b, in]
        # xt[p, n, k, b] = x[b0 + n*CH + b, k*128+p]
        xt = np.ascontiguousarray(
            xc.reshape(NCH, CH, KT, 128).transpose(3, 0, 2, 1)
        )
        in_maps.append(
            {"xt": xt, "wt": wt, "gmt": gmt, "bta": bta, "mut": mut, "mu2": mu2}
        )
    return in_maps


def _gather_out(results, b_total, out_dim, n_cores):
    b_loc = b_total // n_cores
    out = np.empty((b_total, out_dim), dtype=np.float32)
    for c in range(n_cores):
        oc = np.asarray(results[c]["out"]).reshape(out_dim // 128, 128, b_loc)
        out[c * b_loc : (c + 1) * b_loc] = oc.transpose(2, 0, 1).reshape(
            b_loc, out_dim
        )
    return out


def kernel(x, weight, gamma, beta):
    from concourse.bass_utils import run_bass_kernel_spmd

    n_cores = 8
    b_total, in_dim = x.shape
    out_dim = weight.shape[0]

    nc = _get_nc((n_cores, b_total // n_cores, in_dim, out_dim, b_total))
    in_maps = _prepare_in_maps(x, weight, gamma, beta, n_cores)
    res = run_bass_kernel_spmd(nc, in_maps, list(range(n_cores)))
    return _gather_out(res.results, b_total, out_dim, n_cores)


# revision 9
# speedup vs baseline: 1.2740x; 1.2424x over previous
"""Binarized linear block (y = relu(batchnorm(x @ sign(W).T))) on 8 TRN2 NeuronCores.

Strategy:
  - Data-parallel shard of the batch dim (16384 -> 2048 rows/core).
  - Weights binarized + transposed + tiled on host, replicated to all cores.
    Mixed precision on the contraction: the first KBN_N8 k-tiles (of 16) run
    as fp8e4m3 DoubleRow matmuls (2 k-tiles per PE pass), the rest as fp16.
    sign(W) is exact in both dtypes; only x's fp8 rounding (~2.6% elementwise
    on the fp8 fraction -> ~2.6%*sqrt(n8/16) output rel err) costs accuracy.
  - x is pre-transposed so the contraction dim lies on SBUF partitions; the
    matmul computes y^T tiles [o x b] so the BN batch statistics are
    per-partition free-dim reductions.
  - The BN batch MEAN is computed exactly on the host from the quantized x
    (colsum @ sign(W) is cheap) and shipped as an input; only the per-channel
    second moment needs the cross-core exchange.
  - Sync-BN second moments go through tiny per-group AllGathers over DRAM
    bounce buffers, software-pipelined against the matmul stream.  Bounce /
    gather DMAs ride the Scalar HWDGE queue, order-pinned behind the newest
    PSUM-drain so the Tile scheduler can never hoist a collective-dependent
    wait ahead of stream work (the strict-FIFO queues otherwise head-of-line
    block); stat math runs on VectorE, also pinned.
  - Startup: first weight tile and first x chunks are DMA'd in k-slices so
    the first matmul is gated on <1MB.
  - Output is written as y^T tiles [m, 128, b_loc]; host transposes back.
"""

import os as _os

import numpy as np

_BN_EPS = 1e-5

# number of k-tiles (of in_dim/128) computed via fp8 DoubleRow; even.
_N8 = int(_os.environ.get("KBN_N8", "8"))

_CACHE = {}


def _env(name):
    return bool(_os.environ.get(name))


def _group_sizes(mt):
    # Pipelined sync-BN in small uniform groups. Each group's collective is
    # triggered right after its matmuls; its finish phase is emitted two
    # collectives later, so the collective has ~2 group-times of matmul
    # compute to land before any engine reaches the finish instructions.
    if mt <= 2:
        return [mt]
    if mt <= 4:
        return [mt - 2, 1, 1]
    rest = mt - 2
    return [2] * (rest // 2) + ([1] if rest % 2 else []) + [1, 1]


def _build(n_cores, b_loc, in_dim, out_dim, b_total):
    import concourse.bass as bass  # noqa: F401
    import concourse.mybir as mybir
    import concourse.tile as tile
    from concourse import bacc

    f16 = mybir.dt.bfloat16 if _env("KBN_BF16") else mybir.dt.float16
    f32 = mybir.dt.float32
    f8 = mybir.dt.float8e4
    AF = mybir.ActivationFunctionType
    ALU = mybir.AluOpType
    PM = mybir.MatmulPerfMode

    KT = in_dim // 128   # k tiles (contraction)
    MT = out_dim // 128  # output-channel tiles
    CH = min(512, b_loc)  # moving-operand chunk
    NCH = b_loc // CH    # batch chunks
    n8 = max(0, min(_N8, KT)) // 2 * 2
    T8 = n8 // 2         # fp8 DoubleRow k-tile pairs
    K16 = KT - n8        # fp16 k tiles
    groups = _group_sizes(MT)

    nc = bacc.Bacc(
        "TRN2",
        target_bir_lowering=False,
        debug=False,
        enable_asserts=False,
        num_devices=n_cores,
    )

    # layouts put the contraction dim on partitions; batch-chunk DMAs are
    # contiguous per partition
    xt = wt = x8 = w8 = None
    if K16:
        xt = nc.dram_tensor("xt", [128, NCH, K16, CH], f16, kind="ExternalInput")
        wt = nc.dram_tensor("wt", [MT, 128, K16, 128], f16, kind="ExternalInput")
    if T8:
        x8 = nc.dram_tensor("x8", [128, NCH, T8, 2, CH], f8, kind="ExternalInput")
        w8 = nc.dram_tensor("w8", [MT, 128, T8, 2, 128], f8, kind="ExternalInput")
    gmt = nc.dram_tensor("gmt", [128, MT], f32, kind="ExternalInput")
    bta = nc.dram_tensor("bta", [128, MT], f32, kind="ExternalInput")
    # exact batch mean (host-computed) and its square, in [128, MT] layout
    mut = nc.dram_tensor("mut", [128, MT], f32, kind="ExternalInput")
    mu2 = nc.dram_tensor("mu2", [128, MT], f32, kind="ExternalInput")
    out = nc.dram_tensor("out", [MT, 128, b_loc], f32, kind="ExternalOutput")

    with tile.TileContext(nc) as tc:
        with (
            tc.tile_pool(name="xpool", bufs=1) as xpool,
            tc.tile_pool(name="wpool", bufs=4) as wpool,
            tc.tile_pool(name="ypool", bufs=MT) as ypool,
            tc.tile_pool(name="opool", bufs=4) as opool,
            tc.tile_pool(name="stat", bufs=1) as stat,
            tc.tile_pool(name="gstat", bufs=4) as gstat,
            tc.tile_pool(name="psum", bufs=4, space="PSUM") as psum,
            tc.tile_pool(name="psum2", bufs=2, space="PSUM") as psum2,
            tc.tile_pool(name="dram", bufs=4, space="DRAM") as dram,
        ):
            xt_sb = x8_sb = None
            if K16:
                xt_sb = xpool.tile([128, NCH, K16, CH], f16, name="xt_sb")
            if T8:
                x8_sb = xpool.tile([128, NCH, T8, 2, CH], f8, name="x8_sb")

            # Startup: k-sliced loads so the first matmuls are gated on
            # <1MB.  fp8 pieces first (they run first), then fp16 halves;
            # chunk 1 before the second weight tile.
            wts = []
            w8_0 = w16_0 = None
            if T8:
                w8_0 = wpool.tile([128, T8, 2, 128], f8, tag="w8")
                nc.sync.dma_start(w8_0[:], w8.ap()[0])
                nc.sync.dma_start(x8_sb[:, 0], x8.ap()[:, 0])
            if K16:
                w16_0 = wpool.tile([128, K16, 128], f16, tag="wt")
                kh = max(K16 // 2, 1)
                nc.sync.dma_start(w16_0[:, :kh], wt.ap()[0, :, :kh])
                nc.sync.dma_start(xt_sb[:, 0, :kh], xt.ap()[:, 0, :kh])
                nc.sync.dma_start(w16_0[:, kh:], wt.ap()[0, :, kh:])
                nc.sync.dma_start(xt_sb[:, 0, kh:], xt.ap()[:, 0, kh:])
            if NCH > 1:
                if T8:
                    nc.sync.dma_start(x8_sb[:, 1], x8.ap()[:, 1])
                if K16:
                    nc.sync.dma_start(xt_sb[:, 1], xt.ap()[:, 1])
            wts.append((w8_0, w16_0))
            if MT > 1:
                w8_1 = w16_1 = None
                if T8:
                    w8_1 = wpool.tile([128, T8, 2, 128], f8, tag="w8")
                    nc.sync.dma_start(w8_1[:], w8.ap()[1])
                if K16:
                    w16_1 = wpool.tile([128, K16, 128], f16, tag="wt")
                    nc.sync.dma_start(w16_1[:], wt.ap()[1])
                wts.append((w8_1, w16_1))
            for n in range(2, NCH):
                if T8:
                    nc.sync.dma_start(x8_sb[:, n], x8.ap()[:, n])
                if K16:
                    nc.sync.dma_start(xt_sb[:, n], xt.ap()[:, n])

            gamma_sb = stat.tile([128, MT], f32)
            beta_sb = stat.tile([128, MT], f32)
            mu_sb = stat.tile([128, MT], f32)
            mu2_sb = stat.tile([128, MT], f32)
            nc.gpsimd.dma_start(gamma_sb[:], gmt.ap())
            nc.gpsimd.dma_start(beta_sb[:], bta.ap())
            nc.gpsimd.dma_start(mu_sb[:], mut.ap())
            nc.gpsimd.dma_start(mu2_sb[:], mu2.ap())

            eps_t = stat.tile([128, 1], f32)
            nc.vector.memset(eps_t[:], _BN_EPS)
            # nb = eps - mu^2: lets single-tile groups fold the whole
            # variance computation into the Sqrt activation's bias
            nb_t = stat.tile([128, MT], f32)
            nc.gpsimd.memset(nb_t[:], _BN_EPS)
            nc.gpsimd.tensor_sub(nb_t[:], nb_t[:], mu2_sb[:])

            # identity for PE-based transposes of the tiny stats tensors:
            # a [128, c] SBUF->DRAM DMA is 128 c*4B descriptors (~20us even
            # on HWDGE), while the [c, 128] transposed layout is c 512B
            # descriptors. The two transposes cost ~600ns of PE each.
            from concourse.masks import make_identity

            ident = stat.tile([128, 128], f32)
            make_identity(nc, ident[:])

            yts = [None] * MT
            last_mm = [None]    # most recent matmul instruction
            last_act = [None]   # most recent PSUM-drain ACTIVATE (ScalarE)
            last_stat = [None]  # most recent bn_stats (VectorE)

            def pin(inst, anchor):
                # order-only (no semaphore) same-engine pin: keeps
                # collective-dependent ops from being scheduled ahead of
                # stream work on the strict-FIFO engine queues
                if anchor[0] is not None:
                    tile.add_dep_helper(
                        inst.ins,
                        anchor[0].ins,
                        sync=False,
                        reason="pin collective-dependent op behind stream",
                    )

            def emit_chunk(m, wpair, bns, j, n):
                """One (channel-tile, batch-chunk): matmuls + epilogues."""
                w8_m, w16_m = wpair
                ns = slice(n * CH, (n + 1) * CH)
                ps = psum.tile([128, CH], f32)
                idx, total = 0, T8 + K16
                for t in range(T8):
                    last_mm[0] = nc.tensor.matmul(
                        ps[:],
                        w8_m[:, t],
                        x8_sb[:, n, t],
                        start=(idx == 0),
                        stop=(idx == total - 1),
                        perf_mode=PM.DoubleRow,
                    )
                    idx += 1
                for k in range(K16):
                    last_mm[0] = nc.tensor.matmul(
                        ps[:],
                        w16_m[:, k, :],
                        xt_sb[:, n, k, :],
                        start=(idx == 0),
                        stop=(idx == total - 1),
                    )
                    idx += 1
                # VectorE: batch stats first (PSUM bank is single-port, the
                # two readers serialize -- stats lead the doorbell chain);
                # then ScalarE: fp16 copy of y^T
                last_stat[0] = nc.vector.bn_stats(out=bns[:, j, n, :], in_=ps[:])
                last_act[0] = nc.scalar.activation(yts[m][:, ns], ps[:], AF.Identity)

            def emit_collective(m0, gm, bns):
                """Pack the group's second moments and launch its AllGather."""
                # local (mean, var) per channel tile in the group
                mv = gstat.tile([128, gm, 2], f32, tag="mv")
                for j in range(gm):
                    nc.vector.bn_aggr(out=mv[:, j, :], in_=bns[:, j])

                # per-core second moment m2 = var + mean^2, PE-transposed to
                # [gm, 128] so the bounce DMA is gm big descriptors
                st = gstat.tile([128, gm], f32, tag="st")
                nc.vector.tensor_mul(st[:], mv[:, :, 0], mv[:, :, 0])
                nc.vector.tensor_add(st[:], mv[:, :, 1], st[:])

                psT = psum2.tile([gm, 128], f32, tag="psT")
                nc.tensor.transpose(psT[:], st[:], ident[:])
                stT = gstat.tile([gm, 128], f32, tag="stT")
                nc.vector.tensor_copy(stT[:], psT[:])

                bounce_out = None
                if n_cores > 1:
                    bounce_in = dram.tile([gm, 128], f32, tag="bin")
                    bd = nc.scalar.dma_start(bounce_in[:], stT[:])
                    pin(bd, last_act)
                    # AllGather + local reduce: lower latency than an
                    # AllReduce for latency-dominated tiny messages
                    bounce_out = dram.tile([n_cores, gm, 128], f32, tag="bout")
                    nc.gpsimd.collective_compute(
                        "AllGather",
                        ALU.bypass,
                        replica_groups=[list(range(n_cores))],
                        ins=[bounce_in.opt()],
                        outs=[bounce_out.opt()],
                    )
                return (m0, gm, bounce_out, stT)

            def emit_finish(state, split=1):
                m0, gm, bounce_out, stT = state
                if n_cores > 1:
                    allT = gstat.tile([gm, n_cores, 128], f32, tag="allT")
                    gd = nc.scalar.dma_start(
                        allT[:], bounce_out[:].rearrange("r c f -> c r f")
                    )
                    pin(gd, last_act)
                    w = n_cores
                    while w > 1:
                        w //= 2
                        ad = nc.vector.tensor_add(
                            allT[:, :w], allT[:, :w], allT[:, w : 2 * w]
                        )
                        pin(ad, last_stat)
                    redT = allT[:, 0, :]
                else:
                    redT = stT[:]
                psB = psum2.tile([128, gm], f32, tag="psB")
                back_t = nc.tensor.transpose(psB[:], redT, ident[:gm, :gm])
                # pin behind the newest matmul so this collective-dependent
                # PE op can't head-of-line block the matmul stream
                pin(back_t, last_mm)

                ms = slice(m0, m0 + gm)
                sd_t = gstat.tile([128, gm], f32, tag="sd")
                inv_t = gstat.tile([128, gm], f32, tag="inv")
                scale_t = gstat.tile([128, gm], f32, tag="scale")
                tmp_t = gstat.tile([128, gm], f32, tag="tmp")
                shift_t = gstat.tile([128, gm], f32, tag="shift")
                if gm == 1:
                    # lean path: sd = sqrt(m2/W + (eps - mu^2)) straight from
                    # PSUM, variance folded into the activation bias
                    sq = nc.scalar.activation(
                        sd_t[:],
                        psB[:],
                        AF.Sqrt,
                        bias=nb_t[:, m0 : m0 + 1],
                        scale=1.0 / n_cores,
                    )
                    pin(sq, last_act)
                else:
                    m2g = gstat.tile([128, gm], f32, tag="m2g")
                    cpy = nc.vector.tensor_copy(m2g[:], psB[:])
                    pin(cpy, last_stat)
                    var_t = gstat.tile([128, gm], f32, tag="var")
                    ts = nc.vector.tensor_scalar_mul(var_t[:], m2g[:], 1.0 / n_cores)
                    pin(ts, last_stat)
                    sb = nc.vector.tensor_sub(var_t[:], var_t[:], mu2_sb[:, ms])
                    pin(sb, last_stat)
                    sq = nc.scalar.activation(
                        sd_t[:], var_t[:], AF.Sqrt, bias=eps_t[:]
                    )
                    pin(sq, last_act)
                rc = nc.vector.reciprocal(inv_t[:], sd_t[:])
                pin(rc, last_stat)
                sc = nc.vector.tensor_mul(scale_t[:], gamma_sb[:, ms], inv_t[:])
                pin(sc, last_stat)
                tm = nc.vector.tensor_mul(tmp_t[:], mu_sb[:, ms], scale_t[:])
                pin(tm, last_stat)
                sh = nc.vector.tensor_sub(shift_t[:], beta_sb[:, ms], tmp_t[:])
                pin(sh, last_stat)

                hb = b_loc // split
                for j, m in enumerate(range(m0, m0 + gm)):
                    out_m = opool.tile([128, b_loc], f32)
                    for h in range(split):
                        hs = slice(h * hb, (h + 1) * hb)
                        act = nc.scalar.activation(
                            out_m[:, hs],
                            yts[m][:, hs],
                            AF.Relu,
                            bias=shift_t[:, j : j + 1],
                            scale=scale_t[:, j : j + 1],
                        )
                        pin(act, last_act)
                        nc.sync.dma_start(out.ap()[m, :, hs], out_m[:, hs])

            # Pipeline: batch-chunk-outer within each group. Group g's
            # collective block is emitted after group g+1's first chunk pass;
            # its finish is emitted two collectives later. The last two
            # groups are single-tile with immediately-emitted collectives:
            # the final exposed AllGather is the 1-tile one, with a lean
            # finish critical path.
            states = []
            pend_coll = None
            m0 = 0
            G = len(groups)
            for g, gm in enumerate(groups):
                ms = list(range(m0, m0 + gm))
                bns = gstat.tile([128, gm, NCH, 6], f32, tag="bns")
                wtiles = []
                for m in ms:
                    if m < len(wts):
                        wtiles.append(wts[m])
                    else:
                        w8_m = w16_m = None
                        if T8:
                            w8_m = wpool.tile([128, T8, 2, 128], f8, tag="w8")
                            nc.sync.dma_start(w8_m[:], w8.ap()[m])
                        if K16:
                            w16_m = wpool.tile([128, K16, 128], f16, tag="wt")
                            nc.sync.dma_start(w16_m[:], wt.ap()[m])
                        wtiles.append((w8_m, w16_m))
                    yt_m = ypool.tile([128, b_loc], f16, tag="yt")
                    yts[m] = yt_m
                for n in range(NCH):
                    for j, m in enumerate(ms):
                        emit_chunk(m, wtiles[j], bns, j, n)
                    if n == 0 and pend_coll is not None:
                        states.append(emit_collective(*pend_coll))
                        pend_coll = None
                        if len(states) > 2:
                            emit_finish(states.pop(0))
                if g >= G - 2:
                    states.append(emit_collective(m0, gm, bns))
                    if len(states) > 2:
                        emit_finish(states.pop(0), split=2)
                else:
                    pend_coll = (m0, gm, bns)
                m0 += gm
            if pend_coll is not None:
                states.append(emit_collective(*pend_coll))
            for state in states:
                emit_finish(state, split=2)

    nc.compile()
    return nc


def _get_nc(key):
    if key not in _CACHE:
        _CACHE[key] = _build(*key)
    return _CACHE[key]


def _prepare_in_maps(x, weight, gamma, beta, n_cores):
    import ml_dtypes

    b_total, in_dim = x.shape
    out_dim = weight.shape[0]
    b_loc = b_total // n_cores
    KT = in_dim // 128
    MT = out_dim // 128
    CH = min(512, b_loc)
    NCH = b_loc // CH
    n8 = max(0, min(_N8, KT)) // 2 * 2
    T8 = n8 // 2
    K16 = KT - n8
    split = n8 * 128

    _hdt = ml_dtypes.bfloat16 if _env("KBN_BF16") else np.float16
    f8np = ml_dtypes.float8_e4m3

    # host-side marshalling (binarize / transpose / cast / tile)
    wb = np.where(weight >= 0, np.float32(1.0), np.float32(-1.0))
    common = {
        "gmt": np.ascontiguousarray(gamma.reshape(MT, 128).T.astype(np.float32)),
        "bta": np.ascontiguousarray(beta.reshape(MT, 128).T.astype(np.float32)),
    }
    if n8:
        # w8[m, p, t, i, o] = sign(W)[m*128+o, (2t+i)*128 + p]
        common["w8"] = np.ascontiguousarray(
            wb[:, :split]
            .reshape(MT, 128, T8, 2, 128)
            .transpose(0, 4, 2, 3, 1)
            .astype(f8np)
        )
        x8h = x[:, :split].astype(f8np)
    if K16:
        # wt[m, p, k, o] = sign(W)[m*128+o, (n8+k)*128+p]
        common["wt"] = np.ascontiguousarray(
            wb[:, split:]
            .reshape(MT, 128, K16, 128)
            .transpose(0, 3, 2, 1)
            .astype(_hdt)
        )
        x16h = x[:, split:].astype(_hdt)

    # exact batch mean of the device's y (built from the quantized x the
    # kernel actually uses): mean = colsum(x_quant) @ sign(W)^T / B
    colsum = np.zeros(in_dim, dtype=np.float64)
    if n8:
        colsum[:split] = x8h.astype(np.float64).sum(axis=0)
    if K16:
        colsum[split:] = x16h.astype(np.float64).sum(axis=0)
    mu = (wb.astype(np.float64) @ colsum) / b_total  # [out]
    common["mut"] = np.ascontiguousarray(
        mu.reshape(MT, 128).T.astype(np.float32)
    )
    common["mu2"] = np.ascontiguousarray(
        (mu * mu).reshape(MT, 128).T.astype(np.float32)
    )

    in_maps = []
    for c in range(n_cores):
        bs = slice(c * b_loc, (c + 1) * b_loc)
        im = dict(common)
        if n8:
            # x8[p, n, t, i, b] = xq[b0 + n*CH + b, (2t+i)*128 + p]
            im["x8"] = np.ascontiguousarray(
                x8h[bs].reshape(NCH, CH, T8, 2, 128).transpose(4, 0, 2, 3, 1)
            )
        if K16:
            # xt[p, n, k, b] = xq[b0 + n*CH + b, (n8+k)*128+p]
            im["xt"] = np.ascontiguousarray(
                x16h[bs].reshape(NCH, CH, K16, 128).transpose(3, 0, 2, 1)
            )
        in_maps.append(im)
    return in_maps


def _gather_out(results, b_total, out_dim, n_cores):
    b_loc = b_total // n_cores
    out = np.empty((b_total, out_dim), dtype=np.float32)
    for c in range(n_cores):
        oc = np.asarray(results[c]["out"]).reshape(out_dim // 128, 128, b_loc)
        out[c * b_loc : (c + 1) * b_loc] = oc.transpose(2, 0, 1).reshape(
            b_loc, out_dim
        )
    return out


def kernel(x, weight, gamma, beta):
    from concourse.bass_utils import run_bass_kernel_spmd

    n_cores = 8
    b_total, in_dim = x.shape
    out_dim = weight.shape[0]

    nc = _get_nc((n_cores, b_total // n_cores, in_dim, out_dim, b_total))
    in_maps = _prepare_in_maps(x, weight, gamma, beta, n_cores)
    res = run_bass_kernel_spmd(nc, in_maps, list(range(n_cores)))
    return _gather_out(res.results, b_total, out_dim, n_cores)


# revision 13
# speedup vs baseline: 1.2792x; 1.0041x over previous
"""Binarized linear block (y = relu(batchnorm(x @ sign(W).T))) on 8 TRN2 NeuronCores.

Strategy:
  - Data-parallel shard of the batch dim (16384 -> 2048 rows/core).
  - Weights binarized + transposed + tiled on host, replicated to all cores.
    Mixed precision on the contraction: the first KBN_N8 k-tiles (of 16) run
    as fp8e4m3 DoubleRow matmuls (2 k-tiles per PE pass), the rest as fp16.
    sign(W) is exact in both dtypes; only x's fp8 rounding (~2.6% elementwise
    on the fp8 fraction -> ~2.6%*sqrt(n8/16) output rel err) costs accuracy.
  - x is pre-transposed so the contraction dim lies on SBUF partitions; the
    matmul computes y^T tiles [o x b] so the BN batch statistics are
    per-partition free-dim reductions.
  - The BN batch MEAN is computed exactly on the host from the quantized x
    (colsum @ sign(W) is cheap) and shipped as an input; only the per-channel
    second moment needs the cross-core exchange.
  - Sync-BN second moments go through tiny per-group AllGathers over DRAM
    bounce buffers, software-pipelined against the matmul stream.  Bounce /
    gather DMAs ride the Scalar HWDGE queue, order-pinned behind the newest
    PSUM-drain so the Tile scheduler can never hoist a collective-dependent
    wait ahead of stream work (the strict-FIFO queues otherwise head-of-line
    block); stat math runs on VectorE, also pinned.
  - Startup: first weight tile and first x chunks are DMA'd in k-slices so
    the first matmul is gated on <1MB.
  - Output is written as y^T tiles [m, 128, b_loc]; host transposes back.
"""

import os as _os

import numpy as np

_BN_EPS = 1e-5

# number of k-tiles (of in_dim/128) computed via fp8 DoubleRow; even.
_N8 = int(_os.environ.get("KBN_N8", "8"))

_CACHE = {}


def _env(name):
    return bool(_os.environ.get(name))


def _group_sizes(mt):
    # Pipelined sync-BN in uniform groups of 2: collectives then trigger at
    # a 2-m-tile cadence, comfortably above the ~9-12us AllGather latency,
    # so they never queue on the serial CC stream (tail collectives
    # otherwise inherit the previous one's completion as extra latency).
    if mt <= 2:
        return [mt]
    if mt % 2:
        return [2] * ((mt - 1) // 2) + [1]
    return [2] * (mt // 2)


def _build(n_cores, b_loc, in_dim, out_dim, b_total):
    import concourse.bass as bass  # noqa: F401
    import concourse.mybir as mybir
    import concourse.tile as tile
    from concourse import bacc

    f16 = mybir.dt.bfloat16 if _env("KBN_BF16") else mybir.dt.float16
    f32 = mybir.dt.float32
    f8 = mybir.dt.float8e4
    AF = mybir.ActivationFunctionType
    ALU = mybir.AluOpType
    PM = mybir.MatmulPerfMode

    KT = in_dim // 128   # k tiles (contraction)
    MT = out_dim // 128  # output-channel tiles
    CH = min(512, b_loc)  # moving-operand chunk
    NCH = b_loc // CH    # batch chunks
    n8 = max(0, min(_N8, KT)) // 2 * 2
    T8 = n8 // 2         # fp8 DoubleRow k-tile pairs
    K16 = KT - n8        # fp16 k tiles
    groups = _group_sizes(MT)

    nc = bacc.Bacc(
        "TRN2",
        target_bir_lowering=False,
        debug=False,
        enable_asserts=False,
        num_devices=n_cores,
    )

    # layouts put the contraction dim on partitions; batch-chunk DMAs are
    # contiguous per partition
    xt = wt = x8 = w8 = None
    if K16:
        xt = nc.dram_tensor("xt", [128, NCH, K16, CH], f16, kind="ExternalInput")
        wt = nc.dram_tensor("wt", [MT, 128, K16, 128], f16, kind="ExternalInput")
    if T8:
        x8 = nc.dram_tensor("x8", [128, NCH, T8, 2, CH], f8, kind="ExternalInput")
        w8 = nc.dram_tensor("w8", [MT, 128, T8, 2, 128], f8, kind="ExternalInput")
    gmt = nc.dram_tensor("gmt", [128, MT], f32, kind="ExternalInput")
    bta = nc.dram_tensor("bta", [128, MT], f32, kind="ExternalInput")
    # exact batch mean (host-computed) and its square, in [128, MT] layout
    mut = nc.dram_tensor("mut", [128, MT], f32, kind="ExternalInput")
    mu2 = nc.dram_tensor("mu2", [128, MT], f32, kind="ExternalInput")
    out = nc.dram_tensor("out", [MT, 128, b_loc], f32, kind="ExternalOutput")

    with tile.TileContext(nc) as tc:
        with (
            tc.tile_pool(name="xpool", bufs=1) as xpool,
            tc.tile_pool(name="wpool", bufs=4) as wpool,
            tc.tile_pool(name="ypool", bufs=MT) as ypool,
            tc.tile_pool(name="opool", bufs=4) as opool,
            tc.tile_pool(name="stat", bufs=1) as stat,
            tc.tile_pool(name="gstat", bufs=4) as gstat,
            tc.tile_pool(name="psum", bufs=4, space="PSUM") as psum,
            tc.tile_pool(name="psum2", bufs=2, space="PSUM") as psum2,
            tc.tile_pool(name="dram", bufs=4, space="DRAM") as dram,
        ):
            xt_sb = x8_sb = None
            if K16:
                xt_sb = xpool.tile([128, NCH, K16, CH], f16, name="xt_sb")
            if T8:
                x8_sb = xpool.tile([128, NCH, T8, 2, CH], f8, name="x8_sb")

            # Startup: k-sliced loads so the first matmuls are gated on
            # <1MB.  fp8 pieces first (they run first), then fp16 halves;
            # chunk 1 before the second weight tile.
            wts = []
            w8_0 = w16_0 = None
            if T8:
                w8_0 = wpool.tile([128, T8, 2, 128], f8, tag="w8")
                nc.sync.dma_start(w8_0[:], w8.ap()[0])
                nc.sync.dma_start(x8_sb[:, 0], x8.ap()[:, 0])
            if K16:
                w16_0 = wpool.tile([128, K16, 128], f16, tag="wt")
                kh = max(K16 // 2, 1)
                kq = max(K16 // 4, 1)
                nc.sync.dma_start(w16_0[:, :kh], wt.ap()[0, :, :kh])
                nc.sync.dma_start(xt_sb[:, 0, :kq], xt.ap()[:, 0, :kq])
                nc.sync.dma_start(xt_sb[:, 0, kq:kh], xt.ap()[:, 0, kq:kh])
                nc.sync.dma_start(w16_0[:, kh:], wt.ap()[0, :, kh:])
                nc.sync.dma_start(xt_sb[:, 0, kh : kh + kq], xt.ap()[:, 0, kh : kh + kq])
                nc.sync.dma_start(xt_sb[:, 0, kh + kq :], xt.ap()[:, 0, kh + kq :])
            if NCH > 1:
                if T8:
                    nc.sync.dma_start(x8_sb[:, 1], x8.ap()[:, 1])
                if K16:
                    nc.sync.dma_start(xt_sb[:, 1], xt.ap()[:, 1])
            wts.append((w8_0, w16_0))
            if MT > 1:
                w8_1 = w16_1 = None
                if T8:
                    w8_1 = wpool.tile([128, T8, 2, 128], f8, tag="w8")
                    nc.sync.dma_start(w8_1[:], w8.ap()[1])
                if K16:
                    w16_1 = wpool.tile([128, K16, 128], f16, tag="wt")
                    nc.sync.dma_start(w16_1[:], wt.ap()[1])
                wts.append((w8_1, w16_1))
            for n in range(2, NCH):
                if T8:
                    nc.sync.dma_start(x8_sb[:, n], x8.ap()[:, n])
                if K16:
                    nc.sync.dma_start(xt_sb[:, n], xt.ap()[:, n])

            gamma_sb = stat.tile([128, MT], f32)
            beta_sb = stat.tile([128, MT], f32)
            mu_sb = stat.tile([128, MT], f32)
            mu2_sb = stat.tile([128, MT], f32)
            nc.gpsimd.dma_start(gamma_sb[:], gmt.ap())
            nc.gpsimd.dma_start(beta_sb[:], bta.ap())
            nc.gpsimd.dma_start(mu_sb[:], mut.ap())
            nc.gpsimd.dma_start(mu2_sb[:], mu2.ap())

            eps_t = stat.tile([128, 1], f32)
            nc.vector.memset(eps_t[:], _BN_EPS)
            # nb = eps - mu^2: lets single-tile groups fold the whole
            # variance computation into the Sqrt activation's bias
            nb_t = stat.tile([128, MT], f32)
            nc.gpsimd.memset(nb_t[:], _BN_EPS)
            nc.gpsimd.tensor_sub(nb_t[:], nb_t[:], mu2_sb[:])

            # identity for PE-based transposes of the tiny stats tensors:
            # a [128, c] SBUF->DRAM DMA is 128 c*4B descriptors (~20us even
            # on HWDGE), while the [c, 128] transposed layout is c 512B
            # descriptors. The two transposes cost ~600ns of PE each.
            from concourse.masks import make_identity

            ident = stat.tile([128, 128], f32)
            make_identity(nc, ident[:])

            yts = [None] * MT
            last_mm = [None]    # most recent matmul instruction
            last_act = [None]   # most recent PSUM-drain ACTIVATE (ScalarE)
            last_stat = [None]  # most recent bn_stats (VectorE)

            def pin(inst, anchor):
                # order-only (no semaphore) same-engine pin: keeps
                # collective-dependent ops from being scheduled ahead of
                # stream work on the strict-FIFO engine queues
                if anchor[0] is not None:
                    tile.add_dep_helper(
                        inst.ins,
                        anchor[0].ins,
                        sync=False,
                        reason="pin collective-dependent op behind stream",
                    )

            def emit_chunk(m, wpair, bns, j, n):
                """One (channel-tile, batch-chunk): matmuls + epilogues."""
                w8_m, w16_m = wpair
                ns = slice(n * CH, (n + 1) * CH)
                ps = psum.tile([128, CH], f32)
                idx, total = 0, T8 + K16
                for t in range(T8):
                    last_mm[0] = nc.tensor.matmul(
                        ps[:],
                        w8_m[:, t],
                        x8_sb[:, n, t],
                        start=(idx == 0),
                        stop=(idx == total - 1),
                        perf_mode=PM.DoubleRow,
                    )
                    idx += 1
                for k in range(K16):
                    last_mm[0] = nc.tensor.matmul(
                        ps[:],
                        w16_m[:, k, :],
                        xt_sb[:, n, k, :],
                        start=(idx == 0),
                        stop=(idx == total - 1),
                    )
                    idx += 1
                # VectorE: batch stats first (PSUM bank is single-port, the
                # two readers serialize -- stats lead the doorbell chain);
                # then ScalarE: fp16 copy of y^T
                last_stat[0] = nc.vector.bn_stats(out=bns[:, j, n, :], in_=ps[:])
                last_act[0] = nc.scalar.activation(yts[m][:, ns], ps[:], AF.Identity)

            def emit_collective(m0, gm, bns):
                """Pack the group's second moments and launch its AllGather."""
                # local (mean, var) per channel tile in the group
                mv = gstat.tile([128, gm, 2], f32, tag="mv")
                for j in range(gm):
                    nc.vector.bn_aggr(out=mv[:, j, :], in_=bns[:, j])

                # per-core second moment m2 = var + mean^2, PE-transposed to
                # [gm, 128] so the bounce DMA is gm big descriptors
                st = gstat.tile([128, gm], f32, tag="st")
                nc.vector.tensor_mul(st[:], mv[:, :, 0], mv[:, :, 0])
                nc.vector.tensor_add(st[:], mv[:, :, 1], st[:])

                psT = psum2.tile([gm, 128], f32, tag="psT")
                nc.tensor.transpose(psT[:], st[:], ident[:])
                stT = gstat.tile([gm, 128], f32, tag="stT")
                nc.vector.tensor_copy(stT[:], psT[:])

                bounce_out = None
                if n_cores > 1:
                    bounce_in = dram.tile([gm, 128], f32, tag="bin")
                    bd = nc.scalar.dma_start(bounce_in[:], stT[:])
                    pin(bd, last_act)
                    # AllGather + local reduce: lower latency than an
                    # AllReduce for latency-dominated tiny messages
                    bounce_out = dram.tile([n_cores, gm, 128], f32, tag="bout")
                    nc.gpsimd.collective_compute(
                        "AllGather",
                        ALU.bypass,
                        replica_groups=[list(range(n_cores))],
                        ins=[bounce_in.opt()],
                        outs=[bounce_out.opt()],
                    )
                return (m0, gm, bounce_out, stT)

            def emit_finish(state, split=1):
                m0, gm, bounce_out, stT = state
                if n_cores > 1:
                    allT = gstat.tile([gm, n_cores, 128], f32, tag="allT")
                    gd = nc.scalar.dma_start(
                        allT[:], bounce_out[:].rearrange("r c f -> c r f")
                    )
                    pin(gd, last_act)
                    w = n_cores
                    while w > 1:
                        w //= 2
                        ad = nc.vector.tensor_add(
                            allT[:, :w], allT[:, :w], allT[:, w : 2 * w]
                        )
                        pin(ad, last_stat)
                    redT = allT[:, 0, :]
                else:
                    redT = stT[:]
                psB = psum2.tile([128, gm], f32, tag="psB")
                back_t = nc.tensor.transpose(psB[:], redT, ident[:gm, :gm])
                # pin behind the newest matmul so this collective-dependent
                # PE op can't head-of-line block the matmul stream
                pin(back_t, last_mm)

                ms = slice(m0, m0 + gm)
                sd_t = gstat.tile([128, gm], f32, tag="sd")
                inv_t = gstat.tile([128, gm], f32, tag="inv")
                scale_t = gstat.tile([128, gm], f32, tag="scale")
                tmp_t = gstat.tile([128, gm], f32, tag="tmp")
                shift_t = gstat.tile([128, gm], f32, tag="shift")
                # lean path: sd = sqrt(m2/W + (eps - mu^2)) straight from
                # PSUM, variance folded into the per-column activation bias
                for j in range(gm):
                    sq = nc.scalar.activation(
                        sd_t[:, j : j + 1],
                        psB[:, j : j + 1],
                        AF.Sqrt,
                        bias=nb_t[:, m0 + j : m0 + j + 1],
                        scale=1.0 / n_cores,
                    )
                    pin(sq, last_act)
                rc = nc.vector.reciprocal(inv_t[:], sd_t[:])
                pin(rc, last_stat)
                sc = nc.vector.tensor_mul(scale_t[:], gamma_sb[:, ms], inv_t[:])
                pin(sc, last_stat)
                tm = nc.vector.tensor_mul(tmp_t[:], mu_sb[:, ms], scale_t[:])
                pin(tm, last_stat)
                sh = nc.vector.tensor_sub(shift_t[:], beta_sb[:, ms], tmp_t[:])
                pin(sh, last_stat)

                hb = b_loc // split
                for j, m in enumerate(range(m0, m0 + gm)):
                    out_m = opool.tile([128, b_loc], f32)
                    for h in range(split):
                        hs = slice(h * hb, (h + 1) * hb)
                        act = nc.scalar.activation(
                            out_m[:, hs],
                            yts[m][:, hs],
                            AF.Relu,
                            bias=shift_t[:, j : j + 1],
                            scale=scale_t[:, j : j + 1],
                        )
                        pin(act, last_act)
                        nc.sync.dma_start(out.ap()[m, :, hs], out_m[:, hs])

            # Pipeline: batch-chunk-outer within each group. Group g's
            # collective block is emitted after group g+1's first chunk pass;
            # its finish is emitted two collectives later. The last two
            # groups are single-tile with immediately-emitted collectives:
            # the final exposed AllGather is the 1-tile one, with a lean
            # finish critical path.
            states = []
            pend_coll = None
            m0 = 0
            G = len(groups)
            for g, gm in enumerate(groups):
                ms = list(range(m0, m0 + gm))
                bns = gstat.tile([128, gm, NCH, 6], f32, tag="bns")
                wtiles = []
                for m in ms:
                    if m < len(wts):
                        wtiles.append(wts[m])
                    else:
                        w8_m = w16_m = None
                        if T8:
                            w8_m = wpool.tile([128, T8, 2, 128], f8, tag="w8")
                            nc.sync.dma_start(w8_m[:], w8.ap()[m])
                        if K16:
                            w16_m = wpool.tile([128, K16, 128], f16, tag="wt")
                            nc.sync.dma_start(w16_m[:], wt.ap()[m])
                        wtiles.append((w8_m, w16_m))
                    yt_m = ypool.tile([128, b_loc], f16, tag="yt")
                    yts[m] = yt_m
                for n in range(NCH):
                    for j, m in enumerate(ms):
                        emit_chunk(m, wtiles[j], bns, j, n)
                    if n == 0 and pend_coll is not None:
                        states.append(emit_collective(*pend_coll))
                        pend_coll = None
                        if len(states) > 2:
                            emit_finish(states.pop(0))
                if g == G - 1:
                    states.append(emit_collective(m0, gm, bns))
                    if len(states) > 2:
                        emit_finish(states.pop(0), split=2)
                else:
                    pend_coll = (m0, gm, bns)
                m0 += gm
            if pend_coll is not None:
                states.append(emit_collective(*pend_coll))
            for state in states:
                emit_finish(state, split=2)

    nc.compile()
    return nc


def _get_nc(key):
    if key not in _CACHE:
        _CACHE[key] = _build(*key)
    return _CACHE[key]


def _prepare_in_maps(x, weight, gamma, beta, n_cores):
    import ml_dtypes

    b_total, in_dim = x.shape
    out_dim = weight.shape[0]
    b_loc = b_total // n_cores
    KT = in_dim // 128
    MT = out_dim // 128
    CH = min(512, b_loc)
    NCH = b_loc // CH
    n8 = max(0, min(_N8, KT)) // 2 * 2
    T8 = n8 // 2
    K16 = KT - n8
    split = n8 * 128

    _hdt = ml_dtypes.bfloat16 if _env("KBN_BF16") else np.float16
    f8np = ml_dtypes.float8_e4m3

    # host-side marshalling (binarize / transpose / cast / tile)
    wb = np.where(weight >= 0, np.float32(1.0), np.float32(-1.0))
    common = {
        "gmt": np.ascontiguousarray(gamma.reshape(MT, 128).T.astype(np.float32)),
        "bta": np.ascontiguousarray(beta.reshape(MT, 128).T.astype(np.float32)),
    }
    if n8:
        # w8[m, p, t, i, o] = sign(W)[m*128+o, (2t+i)*128 + p]
        common["w8"] = np.ascontiguousarray(
            wb[:, :split]
            .reshape(MT, 128, T8, 2, 128)
            .transpose(0, 4, 2, 3, 1)
            .astype(f8np)
        )
        x8h = x[:, :split].astype(f8np)
    if K16:
        # wt[m, p, k, o] = sign(W)[m*128+o, (n8+k)*128+p]
        common["wt"] = np.ascontiguousarray(
            wb[:, split:]
            .reshape(MT, 128, K16, 128)
            .transpose(0, 3, 2, 1)
            .astype(_hdt)
        )
        x16h = x[:, split:].astype(_hdt)

    # exact batch mean of the device's y (built from the quantized x the
    # kernel actually uses): mean = colsum(x_quant) @ sign(W)^T / B
    colsum = np.zeros(in_dim, dtype=np.float64)
    if n8:
        colsum[:split] = x8h.astype(np.float64).sum(axis=0)
    if K16:
        colsum[split:] = x16h.astype(np.float64).sum(axis=0)
    mu = (wb.astype(np.float64) @ colsum) / b_total  # [out]
    common["mut"] = np.ascontiguousarray(
        mu.reshape(MT, 128).T.astype(np.float32)
    )
    common["mu2"] = np.ascontiguousarray(
        (mu * mu).reshape(MT, 128).T.astype(np.float32)
    )

    in_maps = []
    for c in range(n_cores):
        bs = slice(c * b_loc, (c + 1) * b_loc)
        im = dict(common)
        if n8:
            # x8[p, n, t, i, b] = xq[b0 + n*CH + b, (2t+i)*128 + p]
            im["x8"] = np.ascontiguousarray(
                x8h[bs].reshape(NCH, CH, T8, 2, 128).transpose(4, 0, 2, 3, 1)
            )
        if K16:
            # xt[p, n, k, b] = xq[b0 + n*CH + b, (n8+k)*128+p]
            im["xt"] = np.ascontiguousarray(
                x16h[bs].reshape(NCH, CH, K16, 128).transpose(3, 0, 2, 1)
            )
        in_maps.append(im)
    return in_maps


def _gather_out(results, b_total, out_dim, n_cores):
    b_loc = b_total // n_cores
    out = np.empty((b_total, out_dim), dtype=np.float32)
    for c in range(n_cores):
        oc = np.asarray(results[c]["out"]).reshape(out_dim // 128, 128, b_loc)
        out[c * b_loc : (c + 1) * b_loc] = oc.transpose(2, 0, 1).reshape(
            b_loc, out_dim
        )
    return out


def kernel(x, weight, gamma, beta):
    from concourse.bass_utils import run_bass_kernel_spmd

    n_cores = 8
    b_total, in_dim = x.shape
    out_dim = weight.shape[0]

    nc = _get_nc((n_cores, b_total // n_cores, in_dim, out_dim, b_total))
    in_maps = _prepare_in_maps(x, weight, gamma, beta, n_cores)
    res = run_bass_kernel_spmd(nc, in_maps, list(range(n_cores)))
    return _gather_out(res.results, b_total, out_dim, n_cores)


# revision 14
# speedup vs baseline: 1.3644x; 1.0666x over previous
"""Binarized linear block (y = relu(batchnorm(x @ sign(W).T))) on 8 TRN2 NeuronCores.

Strategy:
  - Data-parallel shard of the batch dim (16384 -> 2048 rows/core).
  - Weights binarized + transposed + tiled on host, replicated to all cores.
    Mixed precision on the contraction: the first KBN_N8 k-tiles (of 16) run
    as fp8e4m3 DoubleRow matmuls (2 k-tiles per PE pass), the rest as fp16.
    sign(W) is exact in both dtypes; only x's fp8 rounding (~2.6% elementwise
    on the fp8 fraction -> ~2.6%*sqrt(n8/16) output rel err) costs accuracy.
  - x is pre-transposed so the contraction dim lies on SBUF partitions; the
    matmul computes y^T tiles [o x b] so the BN batch statistics are
    per-partition free-dim reductions.
  - The BN batch MEAN is computed exactly on the host from the quantized x
    (colsum @ sign(W) is cheap) and shipped as an input; only the per-channel
    second moment needs the cross-core exchange.
  - Sync-BN second moments go through tiny per-group AllGathers over DRAM
    bounce buffers, software-pipelined against the matmul stream.  Bounce /
    gather DMAs ride the Scalar HWDGE queue, order-pinned behind the newest
    PSUM-drain so the Tile scheduler can never hoist a collective-dependent
    wait ahead of stream work (the strict-FIFO queues otherwise head-of-line
    block); stat math runs on VectorE, also pinned.
  - Startup: first weight tile and first x chunks are DMA'd in k-slices so
    the first matmul is gated on <1MB.
  - Output is written as y^T tiles [m, 128, b_loc]; host transposes back.
"""

import os as _os

import numpy as np

_BN_EPS = 1e-5

# number of k-tiles (of in_dim/128) computed via fp8 DoubleRow; even.
_N8 = int(_os.environ.get("KBN_N8", "8"))

_CACHE = {}


def _env(name):
    return bool(_os.environ.get(name))


def _group_sizes(mt):
    # Pipelined sync-BN in uniform groups of 2: collectives then trigger at
    # a 2-m-tile cadence, comfortably above the ~9-12us AllGather latency,
    # so they never queue on the serial CC stream (tail collectives
    # otherwise inherit the previous one's completion as extra latency).
    if mt <= 2:
        return [mt]
    if mt % 2:
        return [2] * ((mt - 1) // 2) + [1]
    return [2] * (mt // 2)


def _build(n_cores, b_loc, in_dim, out_dim, b_total):
    import concourse.bass as bass  # noqa: F401
    import concourse.mybir as mybir
    import concourse.tile as tile
    from concourse import bacc

    f16 = mybir.dt.bfloat16 if _env("KBN_BF16") else mybir.dt.float16
    f32 = mybir.dt.float32
    f8 = mybir.dt.float8e4
    AF = mybir.ActivationFunctionType
    ALU = mybir.AluOpType
    PM = mybir.MatmulPerfMode

    KT = in_dim // 128   # k tiles (contraction)
    MT = out_dim // 128  # output-channel tiles
    CH = min(512, b_loc)  # moving-operand chunk
    NCH = b_loc // CH    # batch chunks
    n8 = max(0, min(_N8, KT)) // 2 * 2
    T8 = n8 // 2         # fp8 DoubleRow k-tile pairs
    K16 = KT - n8        # fp16 k tiles
    groups = _group_sizes(MT)

    nc = bacc.Bacc(
        "TRN2",
        target_bir_lowering=False,
        debug=False,
        enable_asserts=False,
        num_devices=n_cores,
    )

    # layouts put the contraction dim on partitions; batch-chunk DMAs are
    # contiguous per partition
    xt = wt = x8 = w8 = None
    if K16:
        xt = nc.dram_tensor("xt", [128, NCH, K16, CH], f16, kind="ExternalInput")
        wt = nc.dram_tensor("wt", [MT, 128, K16, 128], f16, kind="ExternalInput")
    if T8:
        x8 = nc.dram_tensor("x8", [128, NCH, T8, 2, CH], f8, kind="ExternalInput")
        w8 = nc.dram_tensor("w8", [MT, 128, T8, 2, 128], f8, kind="ExternalInput")
    gmt = nc.dram_tensor("gmt", [128, MT], f32, kind="ExternalInput")
    bta = nc.dram_tensor("bta", [128, MT], f32, kind="ExternalInput")
    # exact batch mean (host-computed) and its square, in [128, MT] layout
    mut = nc.dram_tensor("mut", [128, MT], f32, kind="ExternalInput")
    mu2 = nc.dram_tensor("mu2", [128, MT], f32, kind="ExternalInput")
    out = nc.dram_tensor("out", [MT, 128, b_loc], f32, kind="ExternalOutput")

    with tile.TileContext(nc) as tc:
        with (
            tc.tile_pool(name="xpool", bufs=1) as xpool,
            tc.tile_pool(name="wpool", bufs=4) as wpool,
            tc.tile_pool(name="ypool", bufs=MT) as ypool,
            tc.tile_pool(name="opool", bufs=4) as opool,
            tc.tile_pool(name="stat", bufs=1) as stat,
            tc.tile_pool(name="gstat", bufs=4) as gstat,
            tc.tile_pool(name="psum", bufs=4, space="PSUM") as psum,
            tc.tile_pool(name="psum2", bufs=2, space="PSUM") as psum2,
            tc.tile_pool(name="dram", bufs=4, space="DRAM") as dram,
        ):
            xt_sb = x8_sb = None
            if K16:
                xt_sb = xpool.tile([128, NCH, K16, CH], f16, name="xt_sb")
            if T8:
                x8_sb = xpool.tile([128, NCH, T8, 2, CH], f8, name="x8_sb")

            # Startup: k-sliced loads so the first matmuls are gated on
            # <1MB.  fp8 pieces first (they run first), then fp16 halves;
            # chunk 1 before the second weight tile.
            wts = []
            w8_0 = w16_0 = None
            if T8:
                w8_0 = wpool.tile([128, T8, 2, 128], f8, tag="w8")
                nc.sync.dma_start(w8_0[:], w8.ap()[0])
                nc.sync.dma_start(x8_sb[:, 0], x8.ap()[:, 0])
            if K16:
                w16_0 = wpool.tile([128, K16, 128], f16, tag="wt")
                kh = max(K16 // 2, 1)
                kq = max(K16 // 4, 1)
                nc.sync.dma_start(w16_0[:, :kh], wt.ap()[0, :, :kh])
                nc.sync.dma_start(xt_sb[:, 0, :kq], xt.ap()[:, 0, :kq])
                nc.sync.dma_start(xt_sb[:, 0, kq:kh], xt.ap()[:, 0, kq:kh])
                nc.sync.dma_start(w16_0[:, kh:], wt.ap()[0, :, kh:])
                nc.sync.dma_start(xt_sb[:, 0, kh : kh + kq], xt.ap()[:, 0, kh : kh + kq])
                nc.sync.dma_start(xt_sb[:, 0, kh + kq :], xt.ap()[:, 0, kh + kq :])
            if NCH > 1:
                if T8:
                    nc.sync.dma_start(x8_sb[:, 1], x8.ap()[:, 1])
                if K16:
                    nc.sync.dma_start(xt_sb[:, 1], xt.ap()[:, 1])
            wts.append((w8_0, w16_0))
            if MT > 1:
                w8_1 = w16_1 = None
                if T8:
                    w8_1 = wpool.tile([128, T8, 2, 128], f8, tag="w8")
                    nc.sync.dma_start(w8_1[:], w8.ap()[1])
                if K16:
                    w16_1 = wpool.tile([128, K16, 128], f16, tag="wt")
                    nc.sync.dma_start(w16_1[:], wt.ap()[1])
                wts.append((w8_1, w16_1))
            for n in range(2, NCH):
                if T8:
                    nc.sync.dma_start(x8_sb[:, n], x8.ap()[:, n])
                if K16:
                    nc.sync.dma_start(xt_sb[:, n], xt.ap()[:, n])

            gamma_sb = stat.tile([128, MT], f32)
            beta_sb = stat.tile([128, MT], f32)
            mu_sb = stat.tile([128, MT], f32)
            mu2_sb = stat.tile([128, MT], f32)
            nc.gpsimd.dma_start(gamma_sb[:], gmt.ap())
            nc.gpsimd.dma_start(beta_sb[:], bta.ap())
            nc.gpsimd.dma_start(mu_sb[:], mut.ap())
            nc.gpsimd.dma_start(mu2_sb[:], mu2.ap())

            eps_t = stat.tile([128, 1], f32)
            nc.vector.memset(eps_t[:], _BN_EPS)
            # nb = eps - mu^2: lets single-tile groups fold the whole
            # variance computation into the Sqrt activation's bias
            nb_t = stat.tile([128, MT], f32)
            nc.gpsimd.memset(nb_t[:], _BN_EPS)
            nc.gpsimd.tensor_sub(nb_t[:], nb_t[:], mu2_sb[:])

            # identity for PE-based transposes of the tiny stats tensors:
            # a [128, c] SBUF->DRAM DMA is 128 c*4B descriptors (~20us even
            # on HWDGE), while the [c, 128] transposed layout is c 512B
            # descriptors. The two transposes cost ~600ns of PE each.
            from concourse.masks import make_identity

            ident = stat.tile([128, 128], f32)
            make_identity(nc, ident[:])

            yts = [None] * MT
            last_mm = [None]    # most recent matmul instruction
            last_act = [None]   # most recent PSUM-drain ACTIVATE (ScalarE)
            last_stat = [None]  # most recent bn_stats (VectorE)

            def pin(inst, anchor):
                # order-only (no semaphore) same-engine pin: keeps
                # collective-dependent ops from being scheduled ahead of
                # stream work on the strict-FIFO engine queues
                if anchor[0] is not None:
                    tile.add_dep_helper(
                        inst.ins,
                        anchor[0].ins,
                        sync=False,
                        reason="pin collective-dependent op behind stream",
                    )

            def emit_chunk(m, wpair, bns, j, n):
                """One (channel-tile, batch-chunk): matmuls + epilogues."""
                w8_m, w16_m = wpair
                ns = slice(n * CH, (n + 1) * CH)
                ps = psum.tile([128, CH], f32)
                idx, total = 0, T8 + K16
                for t in range(T8):
                    last_mm[0] = nc.tensor.matmul(
                        ps[:],
                        w8_m[:, t],
                        x8_sb[:, n, t],
                        start=(idx == 0),
                        stop=(idx == total - 1),
                        perf_mode=PM.DoubleRow,
                    )
                    idx += 1
                for k in range(K16):
                    last_mm[0] = nc.tensor.matmul(
                        ps[:],
                        w16_m[:, k, :],
                        xt_sb[:, n, k, :],
                        start=(idx == 0),
                        stop=(idx == total - 1),
                    )
                    idx += 1
                # VectorE: batch stats first (PSUM bank is single-port, the
                # two readers serialize -- stats lead the doorbell chain);
                # then ScalarE: fp16 copy of y^T
                last_stat[0] = nc.vector.bn_stats(out=bns[:, j, n, :], in_=ps[:])
                last_act[0] = nc.scalar.activation(yts[m][:, ns], ps[:], AF.Identity)

            def emit_collective(m0, gm, bns):
                """Pack the group's second moments and launch its AllGather."""
                # local (mean, var) per channel tile in the group
                mv = gstat.tile([128, gm, 2], f32, tag="mv")
                for j in range(gm):
                    nc.vector.bn_aggr(out=mv[:, j, :], in_=bns[:, j])

                # per-core second moment m2 = var + mean^2, PE-transposed to
                # [gm, 128] so the bounce DMA is gm big descriptors
                st = gstat.tile([128, gm], f32, tag="st")
                nc.vector.tensor_mul(st[:], mv[:, :, 0], mv[:, :, 0])
                nc.vector.tensor_add(st[:], mv[:, :, 1], st[:])

                psT = psum2.tile([gm, 128], f32, tag="psT")
                nc.tensor.transpose(psT[:], st[:], ident[:])
                stT = gstat.tile([gm, 128], f32, tag="stT")
                nc.vector.tensor_copy(stT[:], psT[:])

                bounce_out = None
                if n_cores > 1:
                    bounce_in = dram.tile([gm, 128], f32, tag="bin")
                    bd = nc.scalar.dma_start(bounce_in[:], stT[:])
                    pin(bd, last_act)
                    # AllGather + local reduce: lower latency than an
                    # AllReduce for latency-dominated tiny messages
                    bounce_out = dram.tile([n_cores, gm, 128], f32, tag="bout")
                    nc.gpsimd.collective_compute(
                        "AllGather",
                        ALU.bypass,
                        replica_groups=[list(range(n_cores))],
                        ins=[bounce_in.opt()],
                        outs=[bounce_out.opt()],
                    )
                return (m0, gm, bounce_out, stT)

            def emit_finish(state, split=1):
                m0, gm, bounce_out, stT = state
                if n_cores > 1:
                    allT = gstat.tile([gm, n_cores, 128], f32, tag="allT")
                    gd = nc.scalar.dma_start(
                        allT[:], bounce_out[:].rearrange("r c f -> c r f")
                    )
                    pin(gd, last_act)
                    w = n_cores
                    while w > 1:
                        w //= 2
                        ad = nc.vector.tensor_add(
                            allT[:, :w], allT[:, :w], allT[:, w : 2 * w]
                        )
                        pin(ad, last_stat)
                    redT = allT[:, 0, :]
                else:
                    redT = stT[:]
                psB = psum2.tile([128, gm], f32, tag="psB")
                back_t = nc.tensor.transpose(psB[:], redT, ident[:gm, :gm])
                # pin behind the newest matmul so this collective-dependent
                # PE op can't head-of-line block the matmul stream
                pin(back_t, last_mm)

                ms = slice(m0, m0 + gm)
                sd_t = gstat.tile([128, gm], f32, tag="sd")
                inv_t = gstat.tile([128, gm], f32, tag="inv")
                scale_t = gstat.tile([128, gm], f32, tag="scale")
                tmp_t = gstat.tile([128, gm], f32, tag="tmp")
                shift_t = gstat.tile([128, gm], f32, tag="shift")
                # lean path: sd = sqrt(m2/W + (eps - mu^2)) straight from
                # PSUM, variance folded into the per-column activation bias
                for j in range(gm):
                    sq = nc.scalar.activation(
                        sd_t[:, j : j + 1],
                        psB[:, j : j + 1],
                        AF.Sqrt,
                        bias=nb_t[:, m0 + j : m0 + j + 1],
                        scale=1.0 / n_cores,
                    )
                    pin(sq, last_act)
                rc = nc.vector.reciprocal(inv_t[:], sd_t[:])
                pin(rc, last_stat)
                sc = nc.vector.tensor_mul(scale_t[:], gamma_sb[:, ms], inv_t[:])
                pin(sc, last_stat)
                tm = nc.vector.tensor_mul(tmp_t[:], mu_sb[:, ms], scale_t[:])
                pin(tm, last_stat)
                sh = nc.vector.tensor_sub(shift_t[:], beta_sb[:, ms], tmp_t[:])
                pin(sh, last_stat)

                hb = b_loc // split
                for j, m in enumerate(range(m0, m0 + gm)):
                    out_m = opool.tile([128, b_loc], f32)
                    # in the tail (split>1, 2-tile group) the two tiles'
                    # normalizations run on different engines in parallel:
                    # ScalarE activation for one, DVE mult-add+max for the
                    # other -- halves the serial normalize tail
                    use_dve = split > 1 and gm == 2 and j == 0
                    for h in range(split):
                        hs = slice(h * hb, (h + 1) * hb)
                        if use_dve:
                            v1 = nc.vector.tensor_scalar(
                                out_m[:, hs],
                                yts[m][:, hs],
                                scale_t[:, j : j + 1],
                                shift_t[:, j : j + 1],
                                mybir.AluOpType.mult,
                                mybir.AluOpType.add,
                            )
                            pin(v1, last_stat)
                            v2 = nc.vector.tensor_scalar_max(
                                out_m[:, hs], out_m[:, hs], 0.0
                            )
                            pin(v2, last_stat)
                        else:
                            act = nc.scalar.activation(
                                out_m[:, hs],
                                yts[m][:, hs],
                                AF.Relu,
                                bias=shift_t[:, j : j + 1],
                                scale=scale_t[:, j : j + 1],
                            )
                            pin(act, last_act)
                        nc.sync.dma_start(out.ap()[m, :, hs], out_m[:, hs])

            # Pipeline: batch-chunk-outer within each group. Group g's
            # collective block is emitted after group g+1's first chunk pass;
            # its finish is emitted two collectives later. The last two
            # groups are single-tile with immediately-emitted collectives:
            # the final exposed AllGather is the 1-tile one, with a lean
            # finish critical path.
            states = []
            pend_coll = None
            m0 = 0
            G = len(groups)
            for g, gm in enumerate(groups):
                ms = list(range(m0, m0 + gm))
                bns = gstat.tile([128, gm, NCH, 6], f32, tag="bns")
                wtiles = []
                for m in ms:
                    if m < len(wts):
                        wtiles.append(wts[m])
                    else:
                        w8_m = w16_m = None
                        if T8:
                            w8_m = wpool.tile([128, T8, 2, 128], f8, tag="w8")
                            nc.sync.dma_start(w8_m[:], w8.ap()[m])
                        if K16:
                            w16_m = wpool.tile([128, K16, 128], f16, tag="wt")
                            nc.sync.dma_start(w16_m[:], wt.ap()[m])
                        wtiles.append((w8_m, w16_m))
                    yt_m = ypool.tile([128, b_loc], f16, tag="yt")
                    yts[m] = yt_m
                for n in range(NCH):
                    for j, m in enumerate(ms):
                        emit_chunk(m, wtiles[j], bns, j, n)
                    if n == 0 and pend_coll is not None:
                        states.append(emit_collective(*pend_coll))
                        pend_coll = None
                        if len(states) > 2:
                            emit_finish(states.pop(0))
                if g == G - 1:
                    states.append(emit_collective(m0, gm, bns))
                    if len(states) > 2:
                        emit_finish(states.pop(0), split=2)
                else:
                    pend_coll = (m0, gm, bns)
                m0 += gm
            if pend_coll is not None:
                states.append(emit_collective(*pend_coll))
            for state in states:
                emit_finish(state, split=2)

    nc.compile()
    return nc


def _get_nc(key):
    if key not in _CACHE:
        _CACHE[key] = _build(*key)
    return _CACHE[key]


def _prepare_in_maps(x, weight, gamma, beta, n_cores):
    import ml_dtypes

    b_total, in_dim = x.shape
    out_dim = weight.shape[0]
    b_loc = b_total // n_cores
    KT = in_dim // 128
    MT = out_dim // 128
    CH = min(512, b_loc)
    NCH = b_loc // CH
    n8 = max(0, min(_N8, KT)) // 2 * 2
    T8 = n8 // 2
    K16 = KT - n8
    split = n8 * 128

    _hdt = ml_dtypes.bfloat16 if _env("KBN_BF16") else np.float16
    f8np = ml_dtypes.float8_e4m3

    # host-side marshalling (binarize / transpose / cast / tile)
    wb = np.where(weight >= 0, np.float32(1.0), np.float32(-1.0))
    common = {
        "gmt": np.ascontiguousarray(gamma.reshape(MT, 128).T.astype(np.float32)),
        "bta": np.ascontiguousarray(beta.reshape(MT, 128).T.astype(np.float32)),
    }
    if n8:
        # w8[m, p, t, i, o] = sign(W)[m*128+o, (2t+i)*128 + p]
        common["w8"] = np.ascontiguousarray(
            wb[:, :split]
            .reshape(MT, 128, T8, 2, 128)
            .transpose(0, 4, 2, 3, 1)
            .astype(f8np)
        )
        x8h = x[:, :split].astype(f8np)
    if K16:
        # wt[m, p, k, o] = sign(W)[m*128+o, (n8+k)*128+p]
        common["wt"] = np.ascontiguousarray(
            wb[:, split:]
            .reshape(MT, 128, K16, 128)
            .transpose(0, 3, 2, 1)
            .astype(_hdt)
        )
        x16h = x[:, split:].astype(_hdt)

    # exact batch mean of the device's y (built from the quantized x the
    # kernel actually uses): mean = colsum(x_quant) @ sign(W)^T / B
    colsum = np.zeros(in_dim, dtype=np.float64)
    if n8:
        colsum[:split] = x8h.astype(np.float64).sum(axis=0)
    if K16:
        colsum[split:] = x16h.astype(np.float64).sum(axis=0)
    mu = (wb.astype(np.float64) @ colsum) / b_total  # [out]
    common["mut"] = np.ascontiguousarray(
        mu.reshape(MT, 128).T.astype(np.float32)
    )
    common["mu2"] = np.ascontiguousarray(
        (mu * mu).reshape(MT, 128).T.astype(np.float32)
    )

    in_maps = []
    for c in range(n_cores):
        bs = slice(c * b_loc, (c + 1) * b_loc)
        im = dict(common)
        if n8:
            # x8[p, n, t, i, b] = xq[b0 + n*CH + b, (2t+i)*128 + p]
            im["x8"] = np.ascontiguousarray(
                x8h[bs].reshape(NCH, CH, T8, 2, 128).transpose(4, 0, 2, 3, 1)
            )
        if K16:
            # xt[p, n, k, b] = xq[b0 + n*CH + b, (n8+k)*128+p]
            im["xt"] = np.ascontiguousarray(
                x16h[bs].reshape(NCH, CH, K16, 128).transpose(3, 0, 2, 1)
            )
        in_maps.append(im)
    return in_maps


def _gather_out(results, b_total, out_dim, n_cores):
    b_loc = b_total // n_cores
    out = np.empty((b_total, out_dim), dtype=np.float32)
    for c in range(n_cores):
        oc = np.asarray(results[c]["out"]).reshape(out_dim // 128, 128, b_loc)
        out[c * b_loc : (c + 1) * b_loc] = oc.transpose(2, 0, 1).reshape(
            b_loc, out_dim
        )
    return out


def kernel(x, weight, gamma, beta):
    from concourse.bass_utils import run_bass_kernel_spmd

    n_cores = 8
    b_total, in_dim = x.shape
    out_dim = weight.shape[0]

    nc = _get_nc((n_cores, b_total // n_cores, in_dim, out_dim, b_total))
    in_maps = _prepare_in_maps(x, weight, gamma, beta, n_cores)
    res = run_bass_kernel_spmd(nc, in_maps, list(range(n_cores)))
    return _gather_out(res.results, b_total, out_dim, n_cores)
